# revision 60
# baseline (speedup 1.0000x reference)
"""Two-layer GAT (PyG GATConv semantics) on 8 Trainium2 NeuronCores.

Sharding: dst nodes partitioned into 8 contiguous ranges (graph parallel).
v2b pipeline per core:
  Phase A: compute the full layer-1 feature table h = x @ W1 (bf16, with
           a_src/a_dst scores folded into the same matmul via W1 @ A) for
           ALL nodes, replicated on every core (an AllGather of the 23.6MB
           table is cheaper, but dma_gather from large Shared-space tables
           hangs TRN2).  Rows stored bf16 [ad 8 | h 1024 | as 8 | pad] ->
           htab [10240,1152] in local DRAM, batched 4 tiles per DMA.
  Phase B: for each of the core's 10 dst tiles: gather per-edge source rows
           with dma_gather, compute edge attention (batched), aggregate
           messages + softmax denominators with indicator-matrix matmuls on
           the TensorEngine.  Indicators are host-built; ind_en for ALL
           tiles lives resident in SBUF (one DMA), ind_ne streams per tile.
           Per-tile a_dst scores come from an indirect gather of htab cols
           1032:1040.  Epilogue: normalize, +b1, ELU, transpose, matmul
           with W2_ext -> layer-2 table shard (kept in SBUF + one DMA out).
  AG2:     AllGather of the small layer-2 table (h2 | as2 | ad2).
  Phase D: same gather/aggregate for layer 2 (single head), normalize, +b2.

NOTE on the softmax: the reference's jax.ops.segment_max lowers to a segment
*sum* on this backend, so the executed oracle computes
  alpha = exp(e - S_dst) / (sum(exp(e - S_dst)) + 1e-16),   S = sum(e)
which equals exp(e) / (T + exp(S + ln 1e-16)) with T = sum(exp(e)).
We accumulate both T and S per node and use that denominator.
"""
import numpy as np

import concourse.bass as bass
import concourse.bacc as bacc
import concourse.mybir as mybir
import concourse.tile as tile
from concourse.bass_utils import run_bass_kernel_spmd
from concourse.masks import make_identity
from concourse.tile import TileContext

# Problem constants (hardcoded per the harness contract).
N = 10000
E = 160000
IN = 128
H1, C1 = 8, 128
D1 = H1 * C1          # 1024
OUT = 64
NEG = 0.2             # leaky_relu slope
NCORES = 8
P = 128
NP_PAD = 10240        # padded node count: 80 tiles of 128
TPC = 10              # dst tiles per core
NT_ALL = NP_PAD // P  # 80
NLOC = NP_PAD // NCORES  # 1280

F32 = mybir.dt.float32
BF16 = mybir.dt.bfloat16
I16 = mybir.dt.int16

HW = 1152             # h-table row width (2304B bf16): [h 1024 | as 8 | pad]
SW = 8                # score-table row width: ad scores only
T2R = 128             # layer-2 table row width: [h2 64 | as2 | ad2 | pad]
R2W = OUT + 2         # 66: w2ext cols = [msg 64 | as2 | ad2]
W2F = OUT + 2         # 66: rhs2 cols per chunk = [msg 64 | ee | el]
LN16 = float(np.log(np.float32(1e-16)))


def _wrap16(ix):
    """dma_gather idx layout: position i -> [i % 16, i // 16], the 16-row
    block replicated across the 8 GpSimd cores (128 partitions)."""
    n = ix.shape[0]
    a = ix.reshape(n // 16, 16).T
    return np.tile(a, (8, 1))


def _prep_edges(edge_index):
    """Sort edges (plus self-loops) by dst, bucket into per-dst-tile chunk
    lists padded to a uniform per-tile chunk count K.  Build the gather
    index planes and host-side indicator matrices (batched per-tile)."""
    import ml_dtypes
    src = np.concatenate([edge_index[0], np.arange(N)]).astype(np.int64)
    dst = np.concatenate([edge_index[1], np.arange(N)]).astype(np.int64)
    order = np.argsort(dst, kind="stable")
    src, dst = src[order], dst[order]

    tile_id = (dst // P).astype(np.int64)
    counts = np.bincount(tile_id, minlength=NT_ALL)
    K = int(np.max((counts + P - 1) // P))
    cap = K * P

    starts = np.zeros(NT_ALL + 1, np.int64)
    starts[1:] = np.cumsum(counts)

    src16 = np.zeros((NCORES, P, TPC * K * 8), np.int16)
    dloc = np.full((NCORES, TPC * K, P), -1.0, np.float32)
    for t in range(NT_ALL):
        c, lt = divmod(t, TPC)
        n = counts[t]
        sl = slice(starts[t], starts[t + 1])
        buf_s = np.zeros(cap, np.int16)
        buf_l = np.full(cap, -1.0, np.float32)
        buf_s[:n] = src[sl]
        buf_l[:n] = (dst[sl] - t * P).astype(np.float32)
        src16[c, :, lt * K * 8:(lt + 1) * K * 8] = _wrap16(buf_s)
        dloc[c, lt * K:(lt + 1) * K] = buf_l.reshape(K, P)

    # Host-built indicator matrices, batched per tile.
    #   ind_en[c][p, (lt*K+k)*128 + d] = (dloc[c, lt*K+k, p] == d)
    #   ind_ne[c][lt*128+n, k*128+e]   = (dloc[c, lt*K+k, e] == n)
    iota = np.arange(P, dtype=np.float32)
    ind_en = np.zeros((NCORES, P, TPC * K * P), ml_dtypes.bfloat16)
    ind_ne = np.zeros((NCORES, TPC * P, K * P), ml_dtypes.bfloat16)
    for c in range(NCORES):
        ind = (dloc[c][:, :, None] == iota[None, None, :])  # [TPC*K, e, d]
        # en: [e_part, chunk*128 + d]
        ind_en[c] = ind.transpose(1, 0, 2).reshape(
            P, TPC * K * P).astype(ml_dtypes.bfloat16)
        # ne: [TPC, n, K, e] -> [TPC*128, K*128]
        ine = ind.transpose(0, 2, 1).reshape(TPC, K, P, P)  # [TPC, k, n, e]
        ind_ne[c] = ine.transpose(0, 2, 1, 3).reshape(
            TPC * P, K * P).astype(ml_dtypes.bfloat16)

    return K, src16, ind_en, ind_ne


def _build_program(K):
    import os
    dummy_b = os.environ.get("KVAR_B", "0") == "1"
    dummy_d = os.environ.get("KVAR_D", "0") == "1"
    nc = bacc.Bacc("TRN2", target_bir_lowering=False, debug=False,
                   num_swdge_queues=2)

    xTb_d = nc.declare_dram_parameter("xTb", [IN, NP_PAD], BF16, isOutput=False)
    wext_d = nc.declare_dram_parameter("wext", [IN, D1 + 16], BF16, isOutput=False)
    w2ext_d = nc.declare_dram_parameter("w2ext", [D1, R2W], BF16, isOutput=False)
    b1_d = nc.declare_dram_parameter("b1b", [P, D1], F32, isOutput=False)
    b2_d = nc.declare_dram_parameter("b2b", [P, OUT], F32, isOutput=False)
    ind_en_d = nc.declare_dram_parameter("ind_en", [P, TPC * K * P], BF16,
                                         isOutput=False)
    ind_ne_d = nc.declare_dram_parameter("ind_ne", [TPC * P, K * P], BF16,
                                         isOutput=False)
    src16_d = nc.declare_dram_parameter("src16", [P, TPC * K * 8], I16,
                                        isOutput=False)
    adtidx_d = nc.declare_dram_parameter("adtidx", [P, TPC * 8], I16,
                                         isOutput=False)
    out_d = nc.declare_dram_parameter("out", [NLOC, OUT], F32, isOutput=True)
    kdbg = os.environ.get("KDBG", "0") == "1"
    if kdbg:
        dbg_adt_d = nc.declare_dram_parameter("dbg_adt", [P, H1], F32,
                                              isOutput=True)
        dbg_htab_d = nc.declare_dram_parameter("dbg_htab", [P, 32], F32,
                                               isOutput=True)
        dbg_g_d = nc.declare_dram_parameter("dbg_g", [P, 32], F32,
                                            isOutput=True)

    # parts cap at ~6 chunks (768 rows) per dma_gather call: larger calls
    # (1152+ indices) hang the SWDGE gather ucode on this hardware
    NPART = (K + 7) // 8
    bounds = np.linspace(0, K, NPART + 1).astype(int)
    parts = [(int(bounds[i]), int(bounds[i + 1])) for i in range(NPART)]
    KA = max(b - a for a, b in parts)

    with TileContext(nc) as tc:
        with tc.tile_pool(name="dram", bufs=1, space="DRAM") as dram, \
             tc.tile_pool(name="const", bufs=1) as const:

            htab = dram.tile([NP_PAD, HW], BF16)
            h2loc = dram.tile([NLOC, T2R], BF16)
            tab2 = dram.tile([NP_PAD, T2R], BF16, addr_space="Shared")

            ident = const.tile([P, P], F32)
            make_identity(nc, ident[:])
            identb = const.tile([P, P], BF16)
            nc.vector.tensor_copy(out=identb[:], in_=ident[:])
            src16 = const.tile([P, TPC * K * 8], I16)
            nc.sync.dma_start(out=src16[:], in_=src16_d[:])
            ind_en = const.tile([P, TPC * K * P], BF16)
            nc.sync.dma_start(out=ind_en[:], in_=ind_en_d[:])
            b1_sb = const.tile([P, D1], F32)
            nc.sync.dma_start(out=b1_sb[:], in_=b1_d[:])
            b2_sb = const.tile([P, OUT], F32)
            nc.sync.dma_start(out=b2_sb[:], in_=b2_d[:])
            w2_sb = const.tile([P, 8 * R2W], BF16)
            nc.sync.dma_start(
                out=w2_sb[:].rearrange("p (j n) -> p j n", j=8),
                in_=w2ext_d[:].rearrange("(j p) n -> p j n", p=P),
            )
            lncst = const.tile([P, 1], F32)
            nc.gpsimd.memset(lncst[:], LN16)
            m1cst = const.tile([P, 1], F32)
            nc.gpsimd.memset(m1cst[:], -1.0)
            adtidx = const.tile([P, TPC * 8], I16)
            nc.sync.dma_start(out=adtidx[:], in_=adtidx_d[:])
            # per-node attention-score stashes (filled by phase B)
            ad2_all = const.tile([P, TPC], BF16)
            h2_sb = const.tile([P, TPC * T2R], BF16)

            # ---- Phase A: layer-1 features for ALL nodes (replicated) ----
            GA = 8                       # tiles per store batch
            with nc.named_scope("phA"), \
                 tc.tile_pool(name="pha_sb", bufs=2) as sba, \
                 tc.tile_pool(name="pha_c", bufs=1) as sbac, \
                 tc.tile_pool(name="pha_ps", bufs=2, space="PSUM") as psa:
                wext_sb = sbac.tile([P, D1 + 16], BF16, tag="wext")
                nc.sync.dma_start(out=wext_sb[:], in_=wext_d[:])
                for g in range(NT_ALL // GA):
                    xtb = sba.tile([P, GA * P], BF16, tag="xt")
                    nc.sync.dma_start(out=xtb[:],
                                      in_=xTb_d[:, g * GA * P:(g + 1) * GA * P])
                    hbuf = sba.tile([P, GA * HW], BF16, tag="hbuf")
                    nc.vector.memset(
                        hbuf[:].rearrange("p (j w) -> p j w", w=HW)[:, :, 1040:HW],
                        0.0)
                    for u in range(GA):
                        nt = g * GA + u
                        ph = psa.tile([P, D1 + 16], F32, tag="ph")
                        xs = xtb[:, u * P:(u + 1) * P]
                        nc.tensor.matmul(ph[:, 0:512], lhsT=xs,
                                         rhs=wext_sb[:, 0:512],
                                         start=True, stop=True)
                        nc.tensor.matmul(ph[:, 512:1024], lhsT=xs,
                                         rhs=wext_sb[:, 512:1024],
                                         start=True, stop=True)
                        nc.tensor.matmul(ph[:, 1024:1040], lhsT=xs,
                                         rhs=wext_sb[:, 1024:1040],
                                         start=True, stop=True)
                        o0 = u * HW
                        # split the psum->sbuf cast across DVE and ACT
                        nc.vector.tensor_copy(out=hbuf[:, o0:o0 + 512],
                                              in_=ph[:, 0:512])
                        nc.scalar.activation(hbuf[:, o0 + 512:o0 + 1024],
                                             ph[:, 512:1024],
                                             mybir.ActivationFunctionType.Copy)
                        nc.vector.tensor_copy(out=hbuf[:, o0 + 1024:o0 + 1040],
                                              in_=ph[:, 1024:1040])
                    nc.sync.dma_start(
                        out=htab[g * GA * P:(g + 1) * GA * P, :].rearrange(
                            "(j p) w -> p j w", p=P),
                        in_=hbuf[:].rearrange("p (j w) -> p j w", j=GA))

            # ---- Phase B: layer-1 aggregation + layer-2 table shard ----
            with nc.named_scope("phB"), \
                 tc.tile_pool(name="phb_sb", bufs=2) as sbb, \
                 tc.tile_pool(name="phb_epi", bufs=2) as sbe, \
                 tc.tile_pool(name="phb_ps2", bufs=2, space="PSUM") as psb2, \
                 tc.tile_pool(name="phb_psm", bufs=2, space="PSUM") as psmisc:
                if dummy_b:
                    nc.vector.memset(h2_sb[:], 0.0)
                    nc.vector.memset(ad2_all[:], 0.0)
                for lt in range(TPC if not dummy_b else 0):
                    i0 = lt * K * 8
                    pab = psb2.tile([P, 1024], F32, tag="pab")
                    ps_s = psb2.tile([P, 3 * H1], F32, tag="ps_s")

                    indn = sbb.tile([P, K * P], BF16, tag="indn")
                    nc.sync.dma_start(out=indn[:],
                                      in_=ind_ne_d[lt * P:(lt + 1) * P, :])
                    adt_t = sbb.tile([P, P], BF16, tag="adt")
                    nc.gpsimd.dma_gather(
                        adt_t[:].rearrange("p (k w) -> p k w", w=P),
                        htab[:, 0:P], adtidx[:, lt * 8:(lt + 1) * 8],
                        P, P, P, elem_step=HW, queue_num=lt % 2)
                    adt = adt_t[:, 0:H1]
                    if kdbg and lt == 0:
                        dv = sbb.tile([P, H1], F32, tag="dbg1")
                        nc.vector.tensor_copy(out=dv[:], in_=adt_t[:, 0:H1])
                        nc.sync.dma_start(out=dbg_adt_d[:], in_=dv[:])
                        ht0 = sbb.tile([P, 32], BF16, tag="dbg2")
                        nc.sync.dma_start(out=ht0[:], in_=htab[0:P, 0:32])
                        ht0f = sbb.tile([P, 32], F32, tag="dbg3")
                        nc.vector.tensor_copy(out=ht0f[:], in_=ht0[:])
                        nc.sync.dma_start(out=dbg_htab_d[:], in_=ht0f[:])
                    for pi, (ka, kb) in enumerate(parts):
                        kw = kb - ka
                        g = sbb.tile([P, KA * HW], BF16, tag="g")
                        nc.gpsimd.dma_gather(
                            g[:, 0:kw * HW].rearrange("p (k w) -> p k w", w=HW),
                            htab[:], src16[:, i0 + ka * 8:i0 + kb * 8],
                            kw * P, kw * P, HW, queue_num=pi % 2)
                        gv = g[:, 0:kw * HW].rearrange("p (k w) -> p k w", w=HW)
                        if kdbg and lt == 0 and ka == 0:
                            gf = sbb.tile([P, 32], F32, tag="dbg4")
                            nc.vector.tensor_copy(out=gf[:], in_=g[:, 0:32])
                            nc.sync.dma_start(out=dbg_g_d[:], in_=gf[:])

                        adps = psmisc.tile([P, KA * H1], F32, tag="misc",
                                           name=f"adps_{lt}_{ka}")
                        for k in range(ka, kb):
                            j = k - ka
                            nc.tensor.matmul(adps[:, j * H1:(j + 1) * H1],
                                             lhsT=indn[:, k * P:(k + 1) * P],
                                             rhs=adt,
                                             start=True, stop=True)

                        # es = as_src + ad_dst for all chunks of the part
                        es = sbb.tile([P, KA * H1], F32, tag="es")
                        nc.vector.tensor_tensor(
                            out=es[:, 0:kw * H1].rearrange(
                                "p (k w) -> p k w", w=H1),
                            in0=gv[:, :, H1 + D1:H1 + D1 + H1],
                            in1=adps[:, 0:kw * H1].rearrange(
                                "p (k w) -> p k w", w=H1),
                            op=mybir.AluOpType.add)
                        # el = lrelu(es); ee = exp(el) (contiguous, batched)
                        el = sbb.tile([P, KA * H1], F32, tag="el")
                        nc.vector.tensor_scalar_mul(el[:, 0:kw * H1],
                                                    es[:, 0:kw * H1], NEG)
                        nc.vector.tensor_tensor(
                            out=el[:, 0:kw * H1], in0=el[:, 0:kw * H1],
                            in1=es[:, 0:kw * H1], op=mybir.AluOpType.max)
                        # stt = [el_hi | el_lo | exp(el)] per chunk: ONE
                        # accumulation group per PSUM bank (start= clears the
                        # whole bank's has_written bits, so interleaved groups
                        # in one bank corrupt each other)
                        stt = sbb.tile([P, KA * 3 * H1], BF16, tag="stt")
                        sttv = stt[:, 0:kw * 3 * H1].rearrange(
                            "p (k w) -> p k w", w=3 * H1)
                        elv = el[:, 0:kw * H1].rearrange(
                            "p (k w) -> p k w", w=H1)
                        nc.vector.tensor_copy(out=sttv[:, :, 0:H1], in_=elv)
                        # low part of el (bf16 rounding residual) so the raw
                        # score sums S reach f32 accuracy in PSUM
                        nc.vector.tensor_tensor(
                            out=sttv[:, :, H1:2 * H1], in0=elv,
                            in1=sttv[:, :, 0:H1], op=mybir.AluOpType.subtract)
                        nc.scalar.activation(sttv[:, :, 2 * H1:3 * H1], elv,
                                             mybir.ActivationFunctionType.Exp)
                        # ee16: each exp weight replicated x16 so the msg
                        # multiply below has step-1 APs on BOTH operands
                        # (DVE 2x_1P mode instead of 1x with a 0-step AP)
                        ee16 = sbb.tile([P, KA * H1 * 16], BF16, tag="ee16")
                        e16v = ee16[:, 0:kw * H1 * 16].rearrange(
                            "p (k h i) -> p k h i", h=H1, i=16)
                        nc.vector.tensor_copy(
                            out=e16v,
                            in_=sttv[:, :, 2 * H1:3 * H1].rearrange(
                                "p k (h o) -> p k h o", o=1)
                                .to_broadcast([P, kw, H1, 16]))
                        # msg = h_src * exp, broadcast per head (batched)
                        msg = sbb.tile([P, KA * D1], BF16, tag="msg")
                        nc.vector.tensor_tensor(
                            out=msg[:, 0:kw * D1].rearrange(
                                "p (k h r i) -> p k h r i", h=H1, r=8, i=16),
                            in0=gv[:, :, H1:H1 + D1].rearrange(
                                "p k (h r i) -> p k h r i", h=H1, i=16),
                            in1=e16v.rearrange("p k h (o i) -> p k h o i", o=1)
                                .to_broadcast([P, kw, H1, 8, 16]),
                            op=mybir.AluOpType.mult)

                        for k in range(ka, kb):
                            j = k - ka
                            ind = ind_en[:, (lt * K + k) * P:(lt * K + k + 1) * P]
                            first, last = k == 0, k == K - 1
                            mo = j * D1
                            nc.tensor.matmul(pab[:, 0:512], lhsT=ind,
                                             rhs=msg[:, mo:mo + 512],
                                             start=first, stop=last)
                            nc.tensor.matmul(pab[:, 512:1024], lhsT=ind,
                                             rhs=msg[:, mo + 512:mo + 1024],
                                             start=first, stop=last)
                            nc.tensor.matmul(ps_s[:], lhsT=ind,
                                             rhs=stt[:, j * 3 * H1:
                                                     (j + 1) * 3 * H1],
                                             start=first, stop=last)

                    # epilogue: denom = T + exp(S + ln 1e-16); normalize, bias,
                    # ELU, transpose, W2 matmul
                    shl = sbe.tile([P, 2 * H1], F32, tag="shl")
                    nc.vector.tensor_copy(out=shl[:], in_=ps_s[:, 0:2 * H1])
                    ssum = sbe.tile([P, H1], F32, tag="ssum")
                    nc.vector.tensor_tensor(out=ssum[:], in0=shl[:, 0:H1],
                                            in1=shl[:, H1:2 * H1],
                                            op=mybir.AluOpType.add)
                    dd = sbe.tile([P, H1], F32, tag="dd")
                    nc.scalar.activation(dd[:], ssum[:],
                                         mybir.ActivationFunctionType.Exp,
                                         bias=lncst[:])
                    nc.vector.tensor_tensor(out=dd[:], in0=dd[:],
                                            in1=ps_s[:, 2 * H1:3 * H1],
                                            op=mybir.AluOpType.add)
                    rr = sbe.tile([P, H1], F32, tag="rr")
                    nc.vector.reciprocal(rr[:], dd[:])
                    h1 = sbe.tile([P, D1], F32, tag="h1")
                    nc.vector.tensor_tensor(
                        out=h1[:, 0:512].rearrange("p (h c) -> p h c", h=4),
                        in0=pab[:, 0:512].rearrange("p (h c) -> p h c", h=4),
                        in1=rr[:, 0:4].rearrange("p (h o) -> p h o", o=1)
                            .to_broadcast([P, 4, C1]),
                        op=mybir.AluOpType.mult)
                    nc.vector.tensor_tensor(
                        out=h1[:, 512:1024].rearrange("p (h c) -> p h c", h=4),
                        in0=pab[:, 512:1024].rearrange("p (h c) -> p h c", h=4),
                        in1=rr[:, 4:8].rearrange("p (h o) -> p h o", o=1)
                            .to_broadcast([P, 4, C1]),
                        op=mybir.AluOpType.mult)
                    nc.vector.tensor_tensor(out=h1[:], in0=h1[:], in1=b1_sb[:],
                                            op=mybir.AluOpType.add)
                    # ELU: out = exp(x - relu(x)) + relu(x) - 1
                    hr = sbe.tile([P, D1], F32, tag="hr")
                    nc.scalar.activation(hr[:], h1[:],
                                         mybir.ActivationFunctionType.Relu)
                    hm = sbe.tile([P, D1], F32, tag="hm")
                    nc.vector.tensor_tensor(out=hm[:], in0=h1[:], in1=hr[:],
                                            op=mybir.AluOpType.subtract)
                    he = sbe.tile([P, D1], F32, tag="he")
                    nc.scalar.activation(he[:], hm[:],
                                         mybir.ActivationFunctionType.Exp)
                    nc.vector.tensor_tensor(out=hm[:], in0=he[:], in1=hr[:],
                                            op=mybir.AluOpType.add)
                    heb = sbe.tile([P, D1], BF16, tag="heb")
                    nc.scalar.activation(heb[:], hm[:],
                                         mybir.ActivationFunctionType.Identity,
                                         bias=m1cst[:])
                    # transpose he -> ht [ch, node] slices, all bf16 (copies
                    # split across DVE and ACT to balance engine load)
                    ht = sbe.tile([P, D1], BF16, tag="ht")
                    for j in range(8):
                        pt = psmisc.tile([P, P], BF16, tag="misc",
                                         name=f"pt_{lt}_{j}")
                        nc.tensor.transpose(pt[:], in_=heb[:, j * P:(j + 1) * P],
                                            identity=identb[:])
                        if j % 2 == 0:
                            nc.vector.tensor_copy(out=ht[:, j * P:(j + 1) * P],
                                                  in_=pt[:])
                        else:
                            nc.scalar.activation(
                                ht[:, j * P:(j + 1) * P], pt[:],
                                mybir.ActivationFunctionType.Copy)
                    ph2 = psmisc.tile([P, R2W], F32, tag="misc",
                                      name=f"ph2_{lt}")
                    for j in range(8):
                        nc.tensor.matmul(
                            ph2[:], lhsT=ht[:, j * P:(j + 1) * P],
                            rhs=w2_sb[:].rearrange("p (j n) -> p j n", j=8)[:, j, :],
                            start=(j == 0), stop=(j == 7))
                    t0 = lt * T2R
                    nc.vector.memset(h2_sb[:, t0 + R2W:t0 + T2R], 0.0)
                    nc.vector.tensor_copy(out=h2_sb[:, t0:t0 + R2W], in_=ph2[:])
                    nc.vector.tensor_copy(
                        out=ad2_all[:, lt:lt + 1],
                        in_=ph2[:, OUT + 1:OUT + 2])
                nc.sync.dma_start(
                    out=h2loc[:].rearrange("(j p) w -> p j w", p=P),
                    in_=h2_sb[:].rearrange("p (j w) -> p j w", j=TPC))

            # ---- AG2: AllGather the layer-2 table ----
            with nc.named_scope("AG2"):
                nc.gpsimd.collective_compute(
                    "AllGather", mybir.AluOpType.bypass,
                    replica_groups=[list(range(NCORES))],
                    ins=[h2loc.opt()], outs=[tab2.opt()])

            # ---- Phase D: layer-2 aggregation ----
            with nc.named_scope("phD"), \
                 tc.tile_pool(name="phd_sb", bufs=2) as sbd, \
                 tc.tile_pool(name="phd_ps", bufs=2, space="PSUM") as psd:
                if dummy_d:
                    for lt in range(TPC):
                        z = sbd.tile([P, OUT], F32, tag="z")
                        nc.vector.memset(z[:], 0.0)
                        nc.sync.dma_start(
                            out=out_d[lt * P:(lt + 1) * P, :], in_=z[:])
                for lt in range(TPC if not dummy_d else 0):
                    i0 = lt * K * 8
                    po = psd.tile([P, OUT + 16], F32, tag="po")
                    ad2t8 = sbd.tile([P, 8], BF16, tag="ad2t8")
                    nc.vector.tensor_copy(
                        out=ad2t8[:],
                        in_=ad2_all[:, lt:lt + 1].to_broadcast([P, 8]))
                    indn2 = sbd.tile([P, K * P], BF16, tag="indn2")
                    nc.sync.dma_start(out=indn2[:],
                                      in_=ind_ne_d[lt * P:(lt + 1) * P, :])
                    g2 = sbd.tile([P, K * T2R], BF16, tag="g2")
                    for pi, (ka, kb) in enumerate(parts):
                        nc.gpsimd.dma_gather(
                            g2[:, ka * T2R:kb * T2R].rearrange(
                                "p (k w) -> p k w", w=T2R),
                            tab2[:], src16[:, i0 + ka * 8:i0 + kb * 8],
                            (kb - ka) * P, (kb - ka) * P, T2R,
                            queue_num=pi % 2)
                    ad2ps = psd.tile([P, K * 8], F32, tag="ad2ps")
                    for k in range(K):
                        nc.tensor.matmul(ad2ps[:, k * 8:(k + 1) * 8],
                                         lhsT=indn2[:, k * P:(k + 1) * P],
                                         rhs=ad2t8[:], start=True, stop=True)
                    g2v = g2[:].rearrange("p (k w) -> p k w", w=T2R)
                    # es2 = as2_src + ad2_dst, batched over chunks
                    es2 = sbd.tile([P, K], F32, tag="es2")
                    nc.vector.tensor_tensor(
                        out=es2[:].rearrange("p (k o) -> p k o", o=1),
                        in0=g2v[:, :, OUT:OUT + 1],
                        in1=ad2ps[:].rearrange("p (k w) -> p k w", w=8)[:, :, 0:1],
                        op=mybir.AluOpType.add)
                    # el2 = lrelu(es2); ee2 = exp(el2) (contiguous, batched)
                    el2 = sbd.tile([P, K], F32, tag="el2")
                    nc.vector.tensor_scalar_mul(el2[:], es2[:], NEG)
                    nc.vector.tensor_tensor(out=el2[:], in0=el2[:], in1=es2[:],
                                            op=mybir.AluOpType.max)
                    el2b = sbd.tile([P, K], BF16, tag="el2b")
                    nc.vector.tensor_copy(out=el2b[:], in_=el2[:])
                    ee2 = sbd.tile([P, K], BF16, tag="ee2")
                    nc.scalar.activation(ee2[:], el2[:],
                                         mybir.ActivationFunctionType.Exp)
                    # rhs2 = [msg 64 | ee | el] per chunk (single group)
                    rhs2 = sbd.tile([P, K * W2F], BF16, tag="rhs2")
                    r2v = rhs2[:].rearrange("p (k w) -> p k w", w=W2F)
                    nc.vector.tensor_tensor(
                        out=r2v[:, :, 0:OUT],
                        in0=g2v[:, :, 0:OUT],
                        in1=ee2[:].rearrange("p (k o) -> p k o", o=1)
                            .to_broadcast([P, K, OUT]),
                        op=mybir.AluOpType.mult)
                    nc.vector.tensor_copy(
                        out=r2v[:, :, OUT:OUT + 1],
                        in_=ee2[:].rearrange("p (k o) -> p k o", o=1))
                    nc.vector.tensor_copy(
                        out=r2v[:, :, OUT + 1:OUT + 2],
                        in_=el2b[:].rearrange("p (k o) -> p k o", o=1))
                    for k in range(K):
                        first, last = k == 0, k == K - 1
                        ind = ind_en[:, (lt * K + k) * P:(lt * K + k + 1) * P]
                        nc.tensor.matmul(po[:, 0:W2F], lhsT=ind,
                                         rhs=rhs2[:, k * W2F:(k + 1) * W2F],
                                         start=first, stop=last)
                    dd2 = sbd.tile([P, 1], F32, tag="dd2")
                    nc.scalar.activation(dd2[:], po[:, OUT + 1:OUT + 2],
                                         mybir.ActivationFunctionType.Exp,
                                         bias=lncst[:])
                    nc.vector.tensor_tensor(out=dd2[:], in0=dd2[:],
                                            in1=po[:, OUT:OUT + 1],
                                            op=mybir.AluOpType.add)
                    r2 = sbd.tile([P, 1], F32, tag="r2")
                    nc.vector.reciprocal(r2[:], dd2[:])
                    o_sb = sbd.tile([P, OUT], F32, tag="o_sb")
                    nc.vector.tensor_tensor(
                        out=o_sb[:], in0=po[:, 0:OUT],
                        in1=r2[:].to_broadcast([P, OUT]),
                        op=mybir.AluOpType.mult)
                    nc.vector.tensor_tensor(out=o_sb[:], in0=o_sb[:], in1=b2_sb[:],
                                            op=mybir.AluOpType.add)
                    nc.sync.dma_start(out=out_d[lt * P:(lt + 1) * P, :],
                                      in_=o_sb[:])

    nc.compile()
    return nc


_CACHE = {}
TRACE = False          # set by test.py to capture a neuron-profile trace
LAST_EXEC_NS = None
LAST_RESULTS = None


def kernel(x, edge_index, W1, a_src1, a_dst1, b1, W2, a_src2, a_dst2, b2):
    x = np.asarray(x, np.float32)
    edge_index = np.asarray(edge_index)
    W1 = np.asarray(W1, np.float32)
    a_src1 = np.asarray(a_src1, np.float32)
    a_dst1 = np.asarray(a_dst1, np.float32)
    b1 = np.asarray(b1, np.float32)
    W2 = np.asarray(W2, np.float32)
    a_src2 = np.asarray(a_src2, np.float32)
    a_dst2 = np.asarray(a_dst2, np.float32)
    b2 = np.asarray(b2, np.float32)

    K, src16, ind_en, ind_ne = _prep_edges(edge_index)

    # fold attention vectors into the weight matrices (host-side reparam)
    Asrc = np.zeros((D1, H1), np.float32)
    Adst = np.zeros((D1, H1), np.float32)
    for h in range(H1):
        Asrc[h * C1:(h + 1) * C1, h] = a_src1[h]
        Adst[h * C1:(h + 1) * C1, h] = a_dst1[h]
    # htab row layout [ad 8 | h 1024 | as 8]: ad first so the indirect
    # per-node score gather can use an offset-0 source AP
    wext = np.concatenate([W1 @ Adst, W1, W1 @ Asrc], axis=1)       # [128, 1040]
    w2ext = np.concatenate([W2, W2 @ a_src2[0][:, None],
                            W2 @ a_dst2[0][:, None]], axis=1)        # [1024, 66]

    import ml_dtypes
    xT = np.zeros((IN, NP_PAD), np.float32)
    xT[:, :N] = x.T
    xTb = xT.astype(ml_dtypes.bfloat16)
    wextb = wext.astype(ml_dtypes.bfloat16)
    b1b = np.broadcast_to(b1, (P, D1)).copy()
    b2b = np.broadcast_to(b2, (P, OUT)).copy()
    adtidx = np.empty((NCORES, P, TPC * 8), np.int16)
    for c in range(NCORES):
        for lt in range(TPC):
            nodes = (c * NLOC + lt * P + np.arange(P)).astype(np.int16)
            adtidx[c, :, lt * 8:(lt + 1) * 8] = _wrap16(nodes)

    if K not in _CACHE:
        _CACHE[K] = _build_program(K)
    nc = _CACHE[K]

    in_maps = []
    for c in range(NCORES):
        in_maps.append({
            "xTb": xTb,
            "wext": wextb, "w2ext": w2ext.astype(ml_dtypes.bfloat16), "b1b": b1b, "b2b": b2b,
            "src16": src16[c], "adtidx": adtidx[c],
            "ind_en": np.asarray(ind_en[c]), "ind_ne": np.asarray(ind_ne[c]),
        })
    res = run_bass_kernel_spmd(nc, in_maps, list(range(NCORES)), trace=TRACE)
    global LAST_EXEC_NS, LAST_RESULTS
    LAST_EXEC_NS = res.exec_time_ns
    LAST_RESULTS = res
    out = np.concatenate([res.results[c]["out"] for c in range(NCORES)], axis=0)
    return np.ascontiguousarray(out[:N]).astype(np.float32)


# revision 65
# speedup vs baseline: 1.0893x; 1.0893x over previous
"""Two-layer GAT (PyG GATConv semantics) on 8 Trainium2 NeuronCores.

Sharding: dst nodes partitioned into 8 contiguous ranges (graph parallel).
v2b pipeline per core:
  Phase A: compute the full layer-1 feature table h = x @ W1 (bf16, with
           a_src/a_dst scores folded into the same matmul via W1 @ A) for
           ALL nodes, replicated on every core (an AllGather of the 23.6MB
           table is cheaper, but dma_gather from large Shared-space tables
           hangs TRN2).  Rows stored bf16 [ad 8 | h 1024 | as 8 | pad] ->
           htab [10240,1152] in local DRAM, batched 4 tiles per DMA.
  Phase B: for each of the core's 10 dst tiles: gather per-edge source rows
           with dma_gather, compute edge attention (batched), aggregate
           messages + softmax denominators with indicator-matrix matmuls on
           the TensorEngine.  Indicators are host-built; ind_en for ALL
           tiles lives resident in SBUF (one DMA), ind_ne streams per tile.
           Per-tile a_dst scores come from an indirect gather of htab cols
           1032:1040.  Epilogue: normalize, +b1, ELU, transpose, matmul
           with W2_ext -> layer-2 table shard (kept in SBUF + one DMA out).
  AG2:     AllGather of the small layer-2 table (h2 | as2 | ad2).
  Phase D: same gather/aggregate for layer 2 (single head), normalize, +b2.

NOTE on the softmax: the reference's jax.ops.segment_max lowers to a segment
*sum* on this backend, so the executed oracle computes
  alpha = exp(e - S_dst) / (sum(exp(e - S_dst)) + 1e-16),   S = sum(e)
which equals exp(e) / (T + exp(S + ln 1e-16)) with T = sum(exp(e)).
We accumulate both T and S per node and use that denominator.
"""
import numpy as np

import concourse.bass as bass
import concourse.bacc as bacc
import concourse.mybir as mybir
import concourse.tile as tile
from concourse.bass_utils import run_bass_kernel_spmd
from concourse.masks import make_identity
from concourse.tile import TileContext

# Problem constants (hardcoded per the harness contract).
N = 10000
E = 160000
IN = 128
H1, C1 = 8, 128
D1 = H1 * C1          # 1024
OUT = 64
NEG = 0.2             # leaky_relu slope
NCORES = 8
P = 128
NP_PAD = 10240        # padded node count: 80 tiles of 128
TPC = 10              # dst tiles per core
NT_ALL = NP_PAD // P  # 80
NLOC = NP_PAD // NCORES  # 1280

F32 = mybir.dt.float32
BF16 = mybir.dt.bfloat16
I16 = mybir.dt.int16

HW = 1152             # h-table row width (2304B bf16): [h 1024 | as 8 | pad]
SW = 8                # score-table row width: ad scores only
T2R = 128             # layer-2 table row width: [h2 64 | as2 | ad2 | pad]
R2W = OUT + 2         # 66: w2ext cols = [msg 64 | as2 | ad2]
W2F = OUT + 2         # 66: rhs2 cols per chunk = [msg 64 | ee | el]
LN16 = float(np.log(np.float32(1e-16)))


def _wrap16(ix):
    """dma_gather idx layout: position i -> [i % 16, i // 16], the 16-row
    block replicated across the 8 GpSimd cores (128 partitions)."""
    n = ix.shape[0]
    a = ix.reshape(n // 16, 16).T
    return np.tile(a, (8, 1))


def _prep_edges(edge_index):
    """Sort edges (plus self-loops) by dst, bucket into per-dst-tile chunk
    lists padded to a uniform per-tile chunk count K.  Build the gather
    index planes and host-side indicator matrices (batched per-tile)."""
    import ml_dtypes
    src = np.concatenate([edge_index[0], np.arange(N)]).astype(np.int64)
    dst = np.concatenate([edge_index[1], np.arange(N)]).astype(np.int64)
    order = np.argsort(dst, kind="stable")
    src, dst = src[order], dst[order]

    tile_id = (dst // P).astype(np.int64)
    counts = np.bincount(tile_id, minlength=NT_ALL)
    K = int(np.max((counts + P - 1) // P))
    cap = K * P

    starts = np.zeros(NT_ALL + 1, np.int64)
    starts[1:] = np.cumsum(counts)

    src16 = np.zeros((NCORES, P, TPC * K * 8), np.int16)
    dloc = np.full((NCORES, TPC * K, P), -1.0, np.float32)
    for t in range(NT_ALL):
        c, lt = divmod(t, TPC)
        n = counts[t]
        sl = slice(starts[t], starts[t + 1])
        buf_s = np.zeros(cap, np.int16)
        buf_l = np.full(cap, -1.0, np.float32)
        buf_s[:n] = src[sl]
        buf_l[:n] = (dst[sl] - t * P).astype(np.float32)
        src16[c, :, lt * K * 8:(lt + 1) * K * 8] = _wrap16(buf_s)
        dloc[c, lt * K:(lt + 1) * K] = buf_l.reshape(K, P)

    # Host-built indicator matrices, batched per tile.
    #   ind_en[c][p, (lt*K+k)*128 + d] = (dloc[c, lt*K+k, p] == d)
    #   ind_ne[c][lt*128+n, k*128+e]   = (dloc[c, lt*K+k, e] == n)
    iota = np.arange(P, dtype=np.float32)
    ind_en = np.zeros((NCORES, P, TPC * K * P), ml_dtypes.bfloat16)
    ind_ne = np.zeros((NCORES, TPC * P, K * P), ml_dtypes.bfloat16)
    for c in range(NCORES):
        ind = (dloc[c][:, :, None] == iota[None, None, :])  # [TPC*K, e, d]
        # en: [e_part, chunk*128 + d]
        ind_en[c] = ind.transpose(1, 0, 2).reshape(
            P, TPC * K * P).astype(ml_dtypes.bfloat16)
        # ne: [TPC, n, K, e] -> [TPC*128, K*128]
        ine = ind.transpose(0, 2, 1).reshape(TPC, K, P, P)  # [TPC, k, n, e]
        ind_ne[c] = ine.transpose(0, 2, 1, 3).reshape(
            TPC * P, K * P).astype(ml_dtypes.bfloat16)

    return K, src16, ind_en, ind_ne


def _build_program(K):
    import os
    dummy_b = os.environ.get("KVAR_B", "0") == "1"
    dummy_d = os.environ.get("KVAR_D", "0") == "1"
    nc = bacc.Bacc("TRN2", target_bir_lowering=False, debug=False,
                   num_swdge_queues=2)

    xTb_d = nc.declare_dram_parameter("xTb", [IN, NP_PAD], BF16, isOutput=False)
    wext_d = nc.declare_dram_parameter("wext", [IN, D1 + 16], BF16, isOutput=False)
    w2ext_d = nc.declare_dram_parameter("w2ext", [D1, R2W], BF16, isOutput=False)
    b1_d = nc.declare_dram_parameter("b1b", [P, D1], F32, isOutput=False)
    b2_d = nc.declare_dram_parameter("b2b", [P, OUT], F32, isOutput=False)
    ind_en_d = nc.declare_dram_parameter("ind_en", [P, TPC * K * P], BF16,
                                         isOutput=False)
    ind_ne_d = nc.declare_dram_parameter("ind_ne", [TPC * P, K * P], BF16,
                                         isOutput=False)
    src16_d = nc.declare_dram_parameter("src16", [P, TPC * K * 8], I16,
                                        isOutput=False)
    adtidx_d = nc.declare_dram_parameter("adtidx", [P, TPC * 8], I16,
                                         isOutput=False)
    out_d = nc.declare_dram_parameter("out", [NLOC, OUT], F32, isOutput=True)
    kdbg = os.environ.get("KDBG", "0") == "1"
    if kdbg:
        dbg_adt_d = nc.declare_dram_parameter("dbg_adt", [P, H1], F32,
                                              isOutput=True)
        dbg_htab_d = nc.declare_dram_parameter("dbg_htab", [P, 32], F32,
                                               isOutput=True)
        dbg_g_d = nc.declare_dram_parameter("dbg_g", [P, 32], F32,
                                            isOutput=True)

    # parts cap at ~6 chunks (768 rows) per dma_gather call: larger calls
    # (1152+ indices) hang the SWDGE gather ucode on this hardware
    NPART = (K + 7) // 8
    bounds = np.linspace(0, K, NPART + 1).astype(int)
    parts = [(int(bounds[i]), int(bounds[i + 1])) for i in range(NPART)]
    KA = max(b - a for a, b in parts)

    with TileContext(nc) as tc:
        with tc.tile_pool(name="dram", bufs=1, space="DRAM") as dram, \
             tc.tile_pool(name="const", bufs=1) as const:

            htab = dram.tile([NP_PAD, HW], BF16)
            h2loc = dram.tile([NLOC, T2R], BF16)
            tab2 = dram.tile([NP_PAD, T2R], BF16, addr_space="Shared")

            ident = const.tile([P, P], F32)
            make_identity(nc, ident[:])
            identb = const.tile([P, P], BF16)
            nc.vector.tensor_copy(out=identb[:], in_=ident[:])
            src16 = const.tile([P, TPC * K * 8], I16)
            nc.sync.dma_start(out=src16[:], in_=src16_d[:])
            ind_en = const.tile([P, TPC * K * P], BF16)
            nc.sync.dma_start(out=ind_en[:], in_=ind_en_d[:])
            b1_sb = const.tile([P, D1], F32)
            nc.sync.dma_start(out=b1_sb[:], in_=b1_d[:])
            b2_sb = const.tile([P, OUT], F32)
            nc.sync.dma_start(out=b2_sb[:], in_=b2_d[:])
            w2_sb = const.tile([P, 8 * R2W], BF16)
            nc.sync.dma_start(
                out=w2_sb[:].rearrange("p (j n) -> p j n", j=8),
                in_=w2ext_d[:].rearrange("(j p) n -> p j n", p=P),
            )
            lncst = const.tile([P, 1], F32)
            nc.gpsimd.memset(lncst[:], LN16)
            m1cst = const.tile([P, 1], F32)
            nc.gpsimd.memset(m1cst[:], -1.0)
            adtidx = const.tile([P, TPC * 8], I16)
            nc.sync.dma_start(out=adtidx[:], in_=adtidx_d[:])
            adt_all = const.tile([P, TPC * P], BF16)
            # per-node attention-score stashes (filled by phase B)
            ad2_all = const.tile([P, TPC], BF16)
            h2_sb = const.tile([P, TPC * T2R], BF16)

            # ---- Phase A: layer-1 features for ALL nodes (replicated) ----
            GA = 8                       # tiles per store batch
            with nc.named_scope("phA"), \
                 tc.tile_pool(name="pha_sb", bufs=2) as sba, \
                 tc.tile_pool(name="pha_c", bufs=1) as sbac, \
                 tc.tile_pool(name="pha_ps", bufs=2, space="PSUM") as psa:
                wext_sb = sbac.tile([P, D1 + 16], BF16, tag="wext")
                nc.sync.dma_start(out=wext_sb[:], in_=wext_d[:])
                for g in range(NT_ALL // GA):
                    xtb = sba.tile([P, GA * P], BF16, tag="xt")
                    nc.sync.dma_start(out=xtb[:],
                                      in_=xTb_d[:, g * GA * P:(g + 1) * GA * P])
                    hbuf = sba.tile([P, GA * HW], BF16, tag="hbuf")
                    nc.vector.memset(
                        hbuf[:].rearrange("p (j w) -> p j w", w=HW)[:, :, 1040:HW],
                        0.0)
                    for u in range(GA):
                        nt = g * GA + u
                        ph = psa.tile([P, D1 + 16], F32, tag="ph")
                        xs = xtb[:, u * P:(u + 1) * P]
                        nc.tensor.matmul(ph[:, 0:512], lhsT=xs,
                                         rhs=wext_sb[:, 0:512],
                                         start=True, stop=True)
                        nc.tensor.matmul(ph[:, 512:1024], lhsT=xs,
                                         rhs=wext_sb[:, 512:1024],
                                         start=True, stop=True)
                        nc.tensor.matmul(ph[:, 1024:1040], lhsT=xs,
                                         rhs=wext_sb[:, 1024:1040],
                                         start=True, stop=True)
                        o0 = u * HW
                        # split the psum->sbuf cast across DVE and ACT
                        nc.vector.tensor_copy(out=hbuf[:, o0:o0 + 512],
                                              in_=ph[:, 0:512])
                        nc.scalar.activation(hbuf[:, o0 + 512:o0 + 1024],
                                             ph[:, 512:1024],
                                             mybir.ActivationFunctionType.Copy)
                        nc.vector.tensor_copy(out=hbuf[:, o0 + 1024:o0 + 1040],
                                              in_=ph[:, 1024:1040])
                    nc.sync.dma_start(
                        out=htab[g * GA * P:(g + 1) * GA * P, :].rearrange(
                            "(j p) w -> p j w", p=P),
                        in_=hbuf[:].rearrange("p (j w) -> p j w", j=GA))

            # ---- Phase B: layer-1 aggregation + layer-2 table shard ----
            with nc.named_scope("phB"), \
                 tc.tile_pool(name="phb_sb", bufs=2) as sbb, \
                 tc.tile_pool(name="phb_epi", bufs=2) as sbe, \
                 tc.tile_pool(name="phb_ps2", bufs=2, space="PSUM") as psb2, \
                 tc.tile_pool(name="phb_psm", bufs=2, space="PSUM") as psmisc:
                if dummy_b:
                    nc.vector.memset(h2_sb[:], 0.0)
                    nc.vector.memset(ad2_all[:], 0.0)
                # all 10 tiles' a_dst score rows in two up-front gathers so
                # the per-tile adps matmuls never wait mid-stream
                for h in range(2):
                    nc.gpsimd.dma_gather(
                        adt_all[:, h * 5 * P:(h + 1) * 5 * P].rearrange(
                            "p (k w) -> p k w", w=P),
                        htab[:, 0:P], adtidx[:, h * 40:(h + 1) * 40],
                        5 * P, 5 * P, P, elem_step=HW, queue_num=h)
                for lt in range(TPC if not dummy_b else 0):
                    i0 = lt * K * 8
                    pab = psb2.tile([P, 1024], F32, tag="pab")
                    ps_s = psb2.tile([P, 3 * H1], F32, tag="ps_s")

                    indn = sbb.tile([P, K * P], BF16, tag="indn")
                    nc.sync.dma_start(out=indn[:],
                                      in_=ind_ne_d[lt * P:(lt + 1) * P, :])
                    adt = adt_all[:, lt * P:lt * P + H1]
                    if kdbg and lt == 0:
                        dv = sbb.tile([P, H1], F32, tag="dbg1")
                        nc.vector.tensor_copy(out=dv[:], in_=adt)
                        nc.sync.dma_start(out=dbg_adt_d[:], in_=dv[:])
                        ht0 = sbb.tile([P, 32], BF16, tag="dbg2")
                        nc.sync.dma_start(out=ht0[:], in_=htab[0:P, 0:32])
                        ht0f = sbb.tile([P, 32], F32, tag="dbg3")
                        nc.vector.tensor_copy(out=ht0f[:], in_=ht0[:])
                        nc.sync.dma_start(out=dbg_htab_d[:], in_=ht0f[:])
                    for pi, (ka, kb) in enumerate(parts):
                        kw = kb - ka
                        g = sbb.tile([P, KA * HW], BF16, tag="g", bufs=3)
                        nc.gpsimd.dma_gather(
                            g[:, 0:kw * HW].rearrange("p (k w) -> p k w", w=HW),
                            htab[:], src16[:, i0 + ka * 8:i0 + kb * 8],
                            kw * P, kw * P, HW, queue_num=pi % 2)
                        gv = g[:, 0:kw * HW].rearrange("p (k w) -> p k w", w=HW)
                        if kdbg and lt == 0 and ka == 0:
                            gf = sbb.tile([P, 32], F32, tag="dbg4")
                            nc.vector.tensor_copy(out=gf[:], in_=g[:, 0:32])
                            nc.sync.dma_start(out=dbg_g_d[:], in_=gf[:])

                        adps = psmisc.tile([P, KA * H1], F32, tag="misc",
                                           name=f"adps_{lt}_{ka}")
                        for k in range(ka, kb):
                            j = k - ka
                            nc.tensor.matmul(adps[:, j * H1:(j + 1) * H1],
                                             lhsT=indn[:, k * P:(k + 1) * P],
                                             rhs=adt,
                                             start=True, stop=True)

                        # es = as_src + ad_dst for all chunks of the part
                        es = sbb.tile([P, KA * H1], F32, tag="es")
                        nc.vector.tensor_tensor(
                            out=es[:, 0:kw * H1].rearrange(
                                "p (k w) -> p k w", w=H1),
                            in0=gv[:, :, H1 + D1:H1 + D1 + H1],
                            in1=adps[:, 0:kw * H1].rearrange(
                                "p (k w) -> p k w", w=H1),
                            op=mybir.AluOpType.add)
                        # el = lrelu(es); ee = exp(el) (contiguous, batched)
                        el = sbb.tile([P, KA * H1], F32, tag="el")
                        nc.vector.tensor_scalar_mul(el[:, 0:kw * H1],
                                                    es[:, 0:kw * H1], NEG)
                        nc.vector.tensor_tensor(
                            out=el[:, 0:kw * H1], in0=el[:, 0:kw * H1],
                            in1=es[:, 0:kw * H1], op=mybir.AluOpType.max)
                        # stt = [el_hi | el_lo | exp(el)] per chunk: ONE
                        # accumulation group per PSUM bank (start= clears the
                        # whole bank's has_written bits, so interleaved groups
                        # in one bank corrupt each other)
                        stt = sbb.tile([P, KA * 3 * H1], BF16, tag="stt")
                        sttv = stt[:, 0:kw * 3 * H1].rearrange(
                            "p (k w) -> p k w", w=3 * H1)
                        elv = el[:, 0:kw * H1].rearrange(
                            "p (k w) -> p k w", w=H1)
                        nc.vector.tensor_copy(out=sttv[:, :, 0:H1], in_=elv)
                        # low part of el (bf16 rounding residual) so the raw
                        # score sums S reach f32 accuracy in PSUM
                        nc.vector.tensor_tensor(
                            out=sttv[:, :, H1:2 * H1], in0=elv,
                            in1=sttv[:, :, 0:H1], op=mybir.AluOpType.subtract)
                        nc.scalar.activation(sttv[:, :, 2 * H1:3 * H1], elv,
                                             mybir.ActivationFunctionType.Exp)
                        # ee16: each exp weight replicated x16 so the msg
                        # multiply below has step-1 APs on BOTH operands
                        # (DVE 2x_1P mode instead of 1x with a 0-step AP)
                        ee16 = sbb.tile([P, KA * H1 * 16], BF16, tag="ee16")
                        e16v = ee16[:, 0:kw * H1 * 16].rearrange(
                            "p (k h i) -> p k h i", h=H1, i=16)
                        nc.vector.tensor_copy(
                            out=e16v,
                            in_=sttv[:, :, 2 * H1:3 * H1].rearrange(
                                "p k (h o) -> p k h o", o=1)
                                .to_broadcast([P, kw, H1, 16]))
                        # msg = h_src * exp, broadcast per head (batched)
                        msg = sbb.tile([P, KA * D1], BF16, tag="msg")
                        nc.vector.tensor_tensor(
                            out=msg[:, 0:kw * D1].rearrange(
                                "p (k h r i) -> p k h r i", h=H1, r=8, i=16),
                            in0=gv[:, :, H1:H1 + D1].rearrange(
                                "p k (h r i) -> p k h r i", h=H1, i=16),
                            in1=e16v.rearrange("p k h (o i) -> p k h o i", o=1)
                                .to_broadcast([P, kw, H1, 8, 16]),
                            op=mybir.AluOpType.mult)

                        for k in range(ka, kb):
                            j = k - ka
                            ind = ind_en[:, (lt * K + k) * P:(lt * K + k + 1) * P]
                            first, last = k == 0, k == K - 1
                            mo = j * D1
                            nc.tensor.matmul(pab[:, 0:512], lhsT=ind,
                                             rhs=msg[:, mo:mo + 512],
                                             start=first, stop=last)
                            nc.tensor.matmul(pab[:, 512:1024], lhsT=ind,
                                             rhs=msg[:, mo + 512:mo + 1024],
                                             start=first, stop=last)
                            nc.tensor.matmul(ps_s[:], lhsT=ind,
                                             rhs=stt[:, j * 3 * H1:
                                                     (j + 1) * 3 * H1],
                                             start=first, stop=last)

                    # epilogue: denom = T + exp(S + ln 1e-16); normalize, bias,
                    # ELU, transpose, W2 matmul
                    shl = sbe.tile([P, 2 * H1], F32, tag="shl")
                    nc.vector.tensor_copy(out=shl[:], in_=ps_s[:, 0:2 * H1])
                    ssum = sbe.tile([P, H1], F32, tag="ssum")
                    nc.vector.tensor_tensor(out=ssum[:], in0=shl[:, 0:H1],
                                            in1=shl[:, H1:2 * H1],
                                            op=mybir.AluOpType.add)
                    dd = sbe.tile([P, H1], F32, tag="dd")
                    nc.scalar.activation(dd[:], ssum[:],
                                         mybir.ActivationFunctionType.Exp,
                                         bias=lncst[:])
                    nc.vector.tensor_tensor(out=dd[:], in0=dd[:],
                                            in1=ps_s[:, 2 * H1:3 * H1],
                                            op=mybir.AluOpType.add)
                    rr = sbe.tile([P, H1], F32, tag="rr")
                    nc.vector.reciprocal(rr[:], dd[:])
                    h1 = sbe.tile([P, D1], F32, tag="h1")
                    nc.vector.tensor_tensor(
                        out=h1[:, 0:512].rearrange("p (h c) -> p h c", h=4),
                        in0=pab[:, 0:512].rearrange("p (h c) -> p h c", h=4),
                        in1=rr[:, 0:4].rearrange("p (h o) -> p h o", o=1)
                            .to_broadcast([P, 4, C1]),
                        op=mybir.AluOpType.mult)
                    nc.vector.tensor_tensor(
                        out=h1[:, 512:1024].rearrange("p (h c) -> p h c", h=4),
                        in0=pab[:, 512:1024].rearrange("p (h c) -> p h c", h=4),
                        in1=rr[:, 4:8].rearrange("p (h o) -> p h o", o=1)
                            .to_broadcast([P, 4, C1]),
                        op=mybir.AluOpType.mult)
                    nc.vector.tensor_tensor(out=h1[:], in0=h1[:], in1=b1_sb[:],
                                            op=mybir.AluOpType.add)
                    # ELU: out = exp(x - relu(x)) + relu(x) - 1
                    hr = sbe.tile([P, D1], F32, tag="hr")
                    nc.scalar.activation(hr[:], h1[:],
                                         mybir.ActivationFunctionType.Relu)
                    hm = sbe.tile([P, D1], F32, tag="hm")
                    nc.vector.tensor_tensor(out=hm[:], in0=h1[:], in1=hr[:],
                                            op=mybir.AluOpType.subtract)
                    he = sbe.tile([P, D1], F32, tag="he")
                    nc.scalar.activation(he[:], hm[:],
                                         mybir.ActivationFunctionType.Exp)
                    nc.vector.tensor_tensor(out=hm[:], in0=he[:], in1=hr[:],
                                            op=mybir.AluOpType.add)
                    heb = sbe.tile([P, D1], BF16, tag="heb")
                    nc.scalar.activation(heb[:], hm[:],
                                         mybir.ActivationFunctionType.Identity,
                                         bias=m1cst[:])
                    # transpose he -> ht [ch, node] slices, all bf16 (copies
                    # split across DVE and ACT to balance engine load)
                    ht = sbe.tile([P, D1], BF16, tag="ht")
                    for j in range(8):
                        pt = psmisc.tile([P, P], BF16, tag="misc",
                                         name=f"pt_{lt}_{j}")
                        nc.tensor.transpose(pt[:], in_=heb[:, j * P:(j + 1) * P],
                                            identity=identb[:])
                        if j % 2 == 0:
                            nc.vector.tensor_copy(out=ht[:, j * P:(j + 1) * P],
                                                  in_=pt[:])
                        else:
                            nc.scalar.activation(
                                ht[:, j * P:(j + 1) * P], pt[:],
                                mybir.ActivationFunctionType.Copy)
                    ph2 = psmisc.tile([P, R2W], F32, tag="misc",
                                      name=f"ph2_{lt}")
                    for j in range(8):
                        nc.tensor.matmul(
                            ph2[:], lhsT=ht[:, j * P:(j + 1) * P],
                            rhs=w2_sb[:].rearrange("p (j n) -> p j n", j=8)[:, j, :],
                            start=(j == 0), stop=(j == 7))
                    t0 = lt * T2R
                    nc.vector.memset(h2_sb[:, t0 + R2W:t0 + T2R], 0.0)
                    nc.vector.tensor_copy(out=h2_sb[:, t0:t0 + R2W], in_=ph2[:])
                    nc.vector.tensor_copy(
                        out=ad2_all[:, lt:lt + 1],
                        in_=ph2[:, OUT + 1:OUT + 2])
                nc.sync.dma_start(
                    out=h2loc[:].rearrange("(j p) w -> p j w", p=P),
                    in_=h2_sb[:].rearrange("p (j w) -> p j w", j=TPC))

            # ---- AG2: AllGather the layer-2 table ----
            with nc.named_scope("AG2"):
                nc.gpsimd.collective_compute(
                    "AllGather", mybir.AluOpType.bypass,
                    replica_groups=[list(range(NCORES))],
                    ins=[h2loc.opt()], outs=[tab2.opt()])

            # ---- Phase D: layer-2 aggregation ----
            with nc.named_scope("phD"), \
                 tc.tile_pool(name="phd_sb", bufs=2) as sbd, \
                 tc.tile_pool(name="phd_ps", bufs=2, space="PSUM") as psd:
                if dummy_d:
                    for lt in range(TPC):
                        z = sbd.tile([P, OUT], F32, tag="z")
                        nc.vector.memset(z[:], 0.0)
                        nc.sync.dma_start(
                            out=out_d[lt * P:(lt + 1) * P, :], in_=z[:])
                for lt in range(TPC if not dummy_d else 0):
                    i0 = lt * K * 8
                    po = psd.tile([P, OUT + 16], F32, tag="po")
                    ad2t8 = sbd.tile([P, 8], BF16, tag="ad2t8")
                    nc.vector.tensor_copy(
                        out=ad2t8[:],
                        in_=ad2_all[:, lt:lt + 1].to_broadcast([P, 8]))
                    indn2 = sbd.tile([P, K * P], BF16, tag="indn2")
                    nc.sync.dma_start(out=indn2[:],
                                      in_=ind_ne_d[lt * P:(lt + 1) * P, :])
                    g2 = sbd.tile([P, K * T2R], BF16, tag="g2")
                    for pi, (ka, kb) in enumerate(parts):
                        nc.gpsimd.dma_gather(
                            g2[:, ka * T2R:kb * T2R].rearrange(
                                "p (k w) -> p k w", w=T2R),
                            tab2[:], src16[:, i0 + ka * 8:i0 + kb * 8],
                            (kb - ka) * P, (kb - ka) * P, T2R,
                            queue_num=pi % 2)
                    ad2ps = psd.tile([P, K * 8], F32, tag="ad2ps")
                    for k in range(K):
                        nc.tensor.matmul(ad2ps[:, k * 8:(k + 1) * 8],
                                         lhsT=indn2[:, k * P:(k + 1) * P],
                                         rhs=ad2t8[:], start=True, stop=True)
                    g2v = g2[:].rearrange("p (k w) -> p k w", w=T2R)
                    # es2 = as2_src + ad2_dst, batched over chunks
                    es2 = sbd.tile([P, K], F32, tag="es2")
                    nc.vector.tensor_tensor(
                        out=es2[:].rearrange("p (k o) -> p k o", o=1),
                        in0=g2v[:, :, OUT:OUT + 1],
                        in1=ad2ps[:].rearrange("p (k w) -> p k w", w=8)[:, :, 0:1],
                        op=mybir.AluOpType.add)
                    # el2 = lrelu(es2); ee2 = exp(el2) (contiguous, batched)
                    el2 = sbd.tile([P, K], F32, tag="el2")
                    nc.vector.tensor_scalar_mul(el2[:], es2[:], NEG)
                    nc.vector.tensor_tensor(out=el2[:], in0=el2[:], in1=es2[:],
                                            op=mybir.AluOpType.max)
                    el2b = sbd.tile([P, K], BF16, tag="el2b")
                    nc.vector.tensor_copy(out=el2b[:], in_=el2[:])
                    ee2 = sbd.tile([P, K], BF16, tag="ee2")
                    nc.scalar.activation(ee2[:], el2[:],
                                         mybir.ActivationFunctionType.Exp)
                    # rhs2 = [msg 64 | ee | el] per chunk (single group)
                    rhs2 = sbd.tile([P, K * W2F], BF16, tag="rhs2")
                    r2v = rhs2[:].rearrange("p (k w) -> p k w", w=W2F)
                    nc.vector.tensor_tensor(
                        out=r2v[:, :, 0:OUT],
                        in0=g2v[:, :, 0:OUT],
                        in1=ee2[:].rearrange("p (k o) -> p k o", o=1)
                            .to_broadcast([P, K, OUT]),
                        op=mybir.AluOpType.mult)
                    nc.vector.tensor_copy(
                        out=r2v[:, :, OUT:OUT + 1],
                        in_=ee2[:].rearrange("p (k o) -> p k o", o=1))
                    nc.vector.tensor_copy(
                        out=r2v[:, :, OUT + 1:OUT + 2],
                        in_=el2b[:].rearrange("p (k o) -> p k o", o=1))
                    for k in range(K):
                        first, last = k == 0, k == K - 1
                        ind = ind_en[:, (lt * K + k) * P:(lt * K + k + 1) * P]
                        nc.tensor.matmul(po[:, 0:W2F], lhsT=ind,
                                         rhs=rhs2[:, k * W2F:(k + 1) * W2F],
                                         start=first, stop=last)
                    dd2 = sbd.tile([P, 1], F32, tag="dd2")
                    nc.scalar.activation(dd2[:], po[:, OUT + 1:OUT + 2],
                                         mybir.ActivationFunctionType.Exp,
                                         bias=lncst[:])
                    nc.vector.tensor_tensor(out=dd2[:], in0=dd2[:],
                                            in1=po[:, OUT:OUT + 1],
                                            op=mybir.AluOpType.add)
                    r2 = sbd.tile([P, 1], F32, tag="r2")
                    nc.vector.reciprocal(r2[:], dd2[:])
                    o_sb = sbd.tile([P, OUT], F32, tag="o_sb")
                    nc.vector.tensor_tensor(
                        out=o_sb[:], in0=po[:, 0:OUT],
                        in1=r2[:].to_broadcast([P, OUT]),
                        op=mybir.AluOpType.mult)
                    nc.vector.tensor_tensor(out=o_sb[:], in0=o_sb[:], in1=b2_sb[:],
                                            op=mybir.AluOpType.add)
                    nc.sync.dma_start(out=out_d[lt * P:(lt + 1) * P, :],
                                      in_=o_sb[:])

    nc.compile()
    return nc


_CACHE = {}
TRACE = False          # set by test.py to capture a neuron-profile trace
LAST_EXEC_NS = None
LAST_RESULTS = None


def kernel(x, edge_index, W1, a_src1, a_dst1, b1, W2, a_src2, a_dst2, b2):
    x = np.asarray(x, np.float32)
    edge_index = np.asarray(edge_index)
    W1 = np.asarray(W1, np.float32)
    a_src1 = np.asarray(a_src1, np.float32)
    a_dst1 = np.asarray(a_dst1, np.float32)
    b1 = np.asarray(b1, np.float32)
    W2 = np.asarray(W2, np.float32)
    a_src2 = np.asarray(a_src2, np.float32)
    a_dst2 = np.asarray(a_dst2, np.float32)
    b2 = np.asarray(b2, np.float32)

    K, src16, ind_en, ind_ne = _prep_edges(edge_index)

    # fold attention vectors into the weight matrices (host-side reparam)
    Asrc = np.zeros((D1, H1), np.float32)
    Adst = np.zeros((D1, H1), np.float32)
    for h in range(H1):
        Asrc[h * C1:(h + 1) * C1, h] = a_src1[h]
        Adst[h * C1:(h + 1) * C1, h] = a_dst1[h]
    # htab row layout [ad 8 | h 1024 | as 8]: ad first so the indirect
    # per-node score gather can use an offset-0 source AP
    wext = np.concatenate([W1 @ Adst, W1, W1 @ Asrc], axis=1)       # [128, 1040]
    w2ext = np.concatenate([W2, W2 @ a_src2[0][:, None],
                            W2 @ a_dst2[0][:, None]], axis=1)        # [1024, 66]

    import ml_dtypes
    xT = np.zeros((IN, NP_PAD), np.float32)
    xT[:, :N] = x.T
    xTb = xT.astype(ml_dtypes.bfloat16)
    wextb = wext.astype(ml_dtypes.bfloat16)
    b1b = np.broadcast_to(b1, (P, D1)).copy()
    b2b = np.broadcast_to(b2, (P, OUT)).copy()
    adtidx = np.empty((NCORES, P, TPC * 8), np.int16)
    for c in range(NCORES):
        for lt in range(TPC):
            nodes = (c * NLOC + lt * P + np.arange(P)).astype(np.int16)
            adtidx[c, :, lt * 8:(lt + 1) * 8] = _wrap16(nodes)

    if K not in _CACHE:
        _CACHE[K] = _build_program(K)
    nc = _CACHE[K]

    in_maps = []
    for c in range(NCORES):
        in_maps.append({
            "xTb": xTb,
            "wext": wextb, "w2ext": w2ext.astype(ml_dtypes.bfloat16), "b1b": b1b, "b2b": b2b,
            "src16": src16[c], "adtidx": adtidx[c],
            "ind_en": np.asarray(ind_en[c]), "ind_ne": np.asarray(ind_ne[c]),
        })
    res = run_bass_kernel_spmd(nc, in_maps, list(range(NCORES)), trace=TRACE)
    global LAST_EXEC_NS, LAST_RESULTS
    LAST_EXEC_NS = res.exec_time_ns
    LAST_RESULTS = res
    out = np.concatenate([res.results[c]["out"] for c in range(NCORES)], axis=0)
    return np.ascontiguousarray(out[:N]).astype(np.float32)


# revision 66
# speedup vs baseline: 1.1025x; 1.0122x over previous
"""Two-layer GAT (PyG GATConv semantics) on 8 Trainium2 NeuronCores.

Sharding: dst nodes partitioned into 8 contiguous ranges (graph parallel).
v2b pipeline per core:
  Phase A: compute the full layer-1 feature table h = x @ W1 (bf16, with
           a_src/a_dst scores folded into the same matmul via W1 @ A) for
           ALL nodes, replicated on every core (an AllGather of the 23.6MB
           table is cheaper, but dma_gather from large Shared-space tables
           hangs TRN2).  Rows stored bf16 [ad 8 | h 1024 | as 8 | pad] ->
           htab [10240,1152] in local DRAM, batched 4 tiles per DMA.
  Phase B: for each of the core's 10 dst tiles: gather per-edge source rows
           with dma_gather, compute edge attention (batched), aggregate
           messages + softmax denominators with indicator-matrix matmuls on
           the TensorEngine.  Indicators are host-built; ind_en for ALL
           tiles lives resident in SBUF (one DMA), ind_ne streams per tile.
           Per-tile a_dst scores come from an indirect gather of htab cols
           1032:1040.  Epilogue: normalize, +b1, ELU, transpose, matmul
           with W2_ext -> layer-2 table shard (kept in SBUF + one DMA out).
  AG2:     AllGather of the small layer-2 table (h2 | as2 | ad2).
  Phase D: same gather/aggregate for layer 2 (single head), normalize, +b2.

NOTE on the softmax: the reference's jax.ops.segment_max lowers to a segment
*sum* on this backend, so the executed oracle computes
  alpha = exp(e - S_dst) / (sum(exp(e - S_dst)) + 1e-16),   S = sum(e)
which equals exp(e) / (T + exp(S + ln 1e-16)) with T = sum(exp(e)).
We accumulate both T and S per node and use that denominator.
"""
import numpy as np

import concourse.bass as bass
import concourse.bacc as bacc
import concourse.mybir as mybir
import concourse.tile as tile
from concourse.bass_utils import run_bass_kernel_spmd
from concourse.masks import make_identity
from concourse.tile import TileContext

# Problem constants (hardcoded per the harness contract).
N = 10000
E = 160000
IN = 128
H1, C1 = 8, 128
D1 = H1 * C1          # 1024
OUT = 64
NEG = 0.2             # leaky_relu slope
NCORES = 8
P = 128
NP_PAD = 10240        # padded node count: 80 tiles of 128
TPC = 10              # dst tiles per core
NT_ALL = NP_PAD // P  # 80
NLOC = NP_PAD // NCORES  # 1280

F32 = mybir.dt.float32
BF16 = mybir.dt.bfloat16
I16 = mybir.dt.int16

HW = 1152             # h-table row width (2304B bf16): [h 1024 | as 8 | pad]
SW = 8                # score-table row width: ad scores only
T2R = 128             # layer-2 table row width: [h2 64 | as2 | ad2 | pad]
R2W = OUT + 2         # 66: w2ext cols = [msg 64 | as2 | ad2]
W2F = OUT + 2         # 66: rhs2 cols per chunk = [msg 64 | ee | el]
LN16 = float(np.log(np.float32(1e-16)))


def _wrap16(ix):
    """dma_gather idx layout: position i -> [i % 16, i // 16], the 16-row
    block replicated across the 8 GpSimd cores (128 partitions)."""
    n = ix.shape[0]
    a = ix.reshape(n // 16, 16).T
    return np.tile(a, (8, 1))


def _prep_edges(edge_index):
    """Sort edges (plus self-loops) by dst, bucket into per-dst-tile chunk
    lists padded to a uniform per-tile chunk count K.  Build the gather
    index planes and host-side indicator matrices (batched per-tile)."""
    import ml_dtypes
    src = np.concatenate([edge_index[0], np.arange(N)]).astype(np.int64)
    dst = np.concatenate([edge_index[1], np.arange(N)]).astype(np.int64)
    order = np.argsort(dst, kind="stable")
    src, dst = src[order], dst[order]

    tile_id = (dst // P).astype(np.int64)
    counts = np.bincount(tile_id, minlength=NT_ALL)
    K = int(np.max((counts + P - 1) // P))
    cap = K * P

    starts = np.zeros(NT_ALL + 1, np.int64)
    starts[1:] = np.cumsum(counts)

    src16 = np.zeros((NCORES, P, TPC * K * 8), np.int16)
    dloc = np.full((NCORES, TPC * K, P), -1.0, np.float32)
    for t in range(NT_ALL):
        c, lt = divmod(t, TPC)
        n = counts[t]
        sl = slice(starts[t], starts[t + 1])
        buf_s = np.zeros(cap, np.int16)
        buf_l = np.full(cap, -1.0, np.float32)
        buf_s[:n] = src[sl]
        buf_l[:n] = (dst[sl] - t * P).astype(np.float32)
        src16[c, :, lt * K * 8:(lt + 1) * K * 8] = _wrap16(buf_s)
        dloc[c, lt * K:(lt + 1) * K] = buf_l.reshape(K, P)

    # Host-built indicator matrices, batched per tile.
    #   ind_en[c][p, (lt*K+k)*128 + d] = (dloc[c, lt*K+k, p] == d)
    #   ind_ne[c][lt*128+n, k*128+e]   = (dloc[c, lt*K+k, e] == n)
    iota = np.arange(P, dtype=np.float32)
    ind_en = np.zeros((NCORES, P, TPC * K * P), ml_dtypes.bfloat16)
    ind_ne = np.zeros((NCORES, TPC * P, K * P), ml_dtypes.bfloat16)
    for c in range(NCORES):
        ind = (dloc[c][:, :, None] == iota[None, None, :])  # [TPC*K, e, d]
        # en: [e_part, chunk*128 + d]
        ind_en[c] = ind.transpose(1, 0, 2).reshape(
            P, TPC * K * P).astype(ml_dtypes.bfloat16)
        # ne: [TPC, n, K, e] -> [TPC*128, K*128]
        ine = ind.transpose(0, 2, 1).reshape(TPC, K, P, P)  # [TPC, k, n, e]
        ind_ne[c] = ine.transpose(0, 2, 1, 3).reshape(
            TPC * P, K * P).astype(ml_dtypes.bfloat16)

    return K, src16, ind_en, ind_ne


def _build_program(K):
    import os
    dummy_b = os.environ.get("KVAR_B", "0") == "1"
    dummy_d = os.environ.get("KVAR_D", "0") == "1"
    nc = bacc.Bacc("TRN2", target_bir_lowering=False, debug=False,
                   num_swdge_queues=2)

    xTb_d = nc.declare_dram_parameter("xTb", [IN, NP_PAD], BF16, isOutput=False)
    wext_d = nc.declare_dram_parameter("wext", [IN, D1 + 16], BF16, isOutput=False)
    w2ext_d = nc.declare_dram_parameter("w2ext", [D1, R2W], BF16, isOutput=False)
    b1_d = nc.declare_dram_parameter("b1b", [P, D1], F32, isOutput=False)
    b2_d = nc.declare_dram_parameter("b2b", [P, OUT], F32, isOutput=False)
    ind_en_d = nc.declare_dram_parameter("ind_en", [P, TPC * K * P], BF16,
                                         isOutput=False)
    ind_ne_d = nc.declare_dram_parameter("ind_ne", [TPC * P, K * P], BF16,
                                         isOutput=False)
    src16_d = nc.declare_dram_parameter("src16", [P, TPC * K * 8], I16,
                                        isOutput=False)
    adtidx_d = nc.declare_dram_parameter("adtidx", [P, TPC * 8], I16,
                                         isOutput=False)
    out_d = nc.declare_dram_parameter("out", [NLOC, OUT], F32, isOutput=True)
    kdbg = os.environ.get("KDBG", "0") == "1"
    if kdbg:
        dbg_adt_d = nc.declare_dram_parameter("dbg_adt", [P, H1], F32,
                                              isOutput=True)
        dbg_htab_d = nc.declare_dram_parameter("dbg_htab", [P, 32], F32,
                                               isOutput=True)
        dbg_g_d = nc.declare_dram_parameter("dbg_g", [P, 32], F32,
                                            isOutput=True)

    # parts cap at ~6 chunks (768 rows) per dma_gather call: larger calls
    # (1152+ indices) hang the SWDGE gather ucode on this hardware
    NPART = (K + 7) // 8
    bounds = np.linspace(0, K, NPART + 1).astype(int)
    parts = [(int(bounds[i]), int(bounds[i + 1])) for i in range(NPART)]
    KA = max(b - a for a, b in parts)

    with TileContext(nc) as tc:
        with tc.tile_pool(name="dram", bufs=1, space="DRAM") as dram, \
             tc.tile_pool(name="const", bufs=1) as const:

            htab = dram.tile([NP_PAD, HW], BF16)
            h2loc = dram.tile([NLOC, T2R], BF16)
            tab2 = dram.tile([NP_PAD, T2R], BF16, addr_space="Shared")

            ident = const.tile([P, P], F32)
            make_identity(nc, ident[:])
            identb = const.tile([P, P], BF16)
            nc.vector.tensor_copy(out=identb[:], in_=ident[:])
            src16 = const.tile([P, TPC * K * 8], I16)
            nc.sync.dma_start(out=src16[:], in_=src16_d[:])
            ind_en = const.tile([P, TPC * K * P], BF16)
            nc.sync.dma_start(out=ind_en[:], in_=ind_en_d[:])
            b1_sb = const.tile([P, D1], F32)
            nc.sync.dma_start(out=b1_sb[:], in_=b1_d[:])
            b2_sb = const.tile([P, OUT], F32)
            nc.sync.dma_start(out=b2_sb[:], in_=b2_d[:])
            w2_sb = const.tile([P, 8 * R2W], BF16)
            nc.sync.dma_start(
                out=w2_sb[:].rearrange("p (j n) -> p j n", j=8),
                in_=w2ext_d[:].rearrange("(j p) n -> p j n", p=P),
            )
            lncst = const.tile([P, 1], F32)
            nc.gpsimd.memset(lncst[:], LN16)
            m1cst = const.tile([P, 1], F32)
            nc.gpsimd.memset(m1cst[:], -1.0)
            adtidx = const.tile([P, TPC * 8], I16)
            nc.sync.dma_start(out=adtidx[:], in_=adtidx_d[:])
            adt_all = const.tile([P, TPC * P], BF16)
            # per-node attention-score stashes (filled by phase B)
            ad2_all = const.tile([P, TPC], BF16)
            h2_sb = const.tile([P, TPC * T2R], BF16)

            # ---- Phase A: layer-1 features for ALL nodes (replicated) ----
            GA = 8                       # tiles per store batch
            with nc.named_scope("phA"), \
                 tc.tile_pool(name="pha_sb", bufs=2) as sba, \
                 tc.tile_pool(name="pha_c", bufs=1) as sbac, \
                 tc.tile_pool(name="pha_ps", bufs=2, space="PSUM") as psa:
                wext_sb = sbac.tile([P, D1 + 16], BF16, tag="wext")
                nc.sync.dma_start(out=wext_sb[:], in_=wext_d[:])
                for g in range(NT_ALL // GA):
                    xtb = sba.tile([P, GA * P], BF16, tag="xt")
                    nc.sync.dma_start(out=xtb[:],
                                      in_=xTb_d[:, g * GA * P:(g + 1) * GA * P])
                    hbuf = sba.tile([P, GA * HW], BF16, tag="hbuf")
                    nc.vector.memset(
                        hbuf[:].rearrange("p (j w) -> p j w", w=HW)[:, :, 1040:HW],
                        0.0)
                    for u in range(GA):
                        nt = g * GA + u
                        ph = psa.tile([P, D1 + 16], F32, tag="ph")
                        xs = xtb[:, u * P:(u + 1) * P]
                        nc.tensor.matmul(ph[:, 0:512], lhsT=xs,
                                         rhs=wext_sb[:, 0:512],
                                         start=True, stop=True)
                        nc.tensor.matmul(ph[:, 512:1024], lhsT=xs,
                                         rhs=wext_sb[:, 512:1024],
                                         start=True, stop=True)
                        nc.tensor.matmul(ph[:, 1024:1040], lhsT=xs,
                                         rhs=wext_sb[:, 1024:1040],
                                         start=True, stop=True)
                        o0 = u * HW
                        # split the psum->sbuf cast across DVE and ACT
                        nc.vector.tensor_copy(out=hbuf[:, o0:o0 + 512],
                                              in_=ph[:, 0:512])
                        nc.scalar.activation(hbuf[:, o0 + 512:o0 + 1024],
                                             ph[:, 512:1024],
                                             mybir.ActivationFunctionType.Copy)
                        nc.vector.tensor_copy(out=hbuf[:, o0 + 1024:o0 + 1040],
                                              in_=ph[:, 1024:1040])
                    nc.sync.dma_start(
                        out=htab[g * GA * P:(g + 1) * GA * P, :].rearrange(
                            "(j p) w -> p j w", p=P),
                        in_=hbuf[:].rearrange("p (j w) -> p j w", j=GA))

            # ---- Phase B: layer-1 aggregation + layer-2 table shard ----
            with nc.named_scope("phB"), \
                 tc.tile_pool(name="phb_sb", bufs=2) as sbb, \
                 tc.tile_pool(name="phb_epi", bufs=2) as sbe, \
                 tc.tile_pool(name="phb_ps2", bufs=2, space="PSUM") as psb2, \
                 tc.tile_pool(name="phb_psm", bufs=2, space="PSUM") as psmisc:
                if dummy_b:
                    nc.vector.memset(h2_sb[:], 0.0)
                    nc.vector.memset(ad2_all[:], 0.0)
                # all 10 tiles' a_dst score rows in two up-front gathers so
                # the per-tile adps matmuls never wait mid-stream
                for h in range(2):
                    nc.gpsimd.dma_gather(
                        adt_all[:, h * 5 * P:(h + 1) * 5 * P].rearrange(
                            "p (k w) -> p k w", w=P),
                        htab[:, 0:P], adtidx[:, h * 40:(h + 1) * 40],
                        5 * P, 5 * P, P, elem_step=HW, queue_num=h)
                for lt in range(TPC if not dummy_b else 0):
                    i0 = lt * K * 8
                    pab = psb2.tile([P, 1024], F32, tag="pab")
                    ps_s = psb2.tile([P, 3 * H1], F32, tag="ps_s")

                    indn = sbb.tile([P, K * P], BF16, tag="indn")
                    nc.sync.dma_start(out=indn[:],
                                      in_=ind_ne_d[lt * P:(lt + 1) * P, :])
                    adt = adt_all[:, lt * P:lt * P + H1]
                    if kdbg and lt == 0:
                        dv = sbb.tile([P, H1], F32, tag="dbg1")
                        nc.vector.tensor_copy(out=dv[:], in_=adt)
                        nc.sync.dma_start(out=dbg_adt_d[:], in_=dv[:])
                        ht0 = sbb.tile([P, 32], BF16, tag="dbg2")
                        nc.sync.dma_start(out=ht0[:], in_=htab[0:P, 0:32])
                        ht0f = sbb.tile([P, 32], F32, tag="dbg3")
                        nc.vector.tensor_copy(out=ht0f[:], in_=ht0[:])
                        nc.sync.dma_start(out=dbg_htab_d[:], in_=ht0f[:])
                    for pi, (ka, kb) in enumerate(parts):
                        kw = kb - ka
                        g = sbb.tile([P, KA * HW], BF16, tag="g", bufs=3)
                        nc.gpsimd.dma_gather(
                            g[:, 0:kw * HW].rearrange("p (k w) -> p k w", w=HW),
                            htab[:], src16[:, i0 + ka * 8:i0 + kb * 8],
                            kw * P, kw * P, HW, queue_num=pi % 2)
                        gv = g[:, 0:kw * HW].rearrange("p (k w) -> p k w", w=HW)
                        if kdbg and lt == 0 and ka == 0:
                            gf = sbb.tile([P, 32], F32, tag="dbg4")
                            nc.vector.tensor_copy(out=gf[:], in_=g[:, 0:32])
                            nc.sync.dma_start(out=dbg_g_d[:], in_=gf[:])

                        adps = psmisc.tile([P, KA * H1], F32, tag="misc",
                                           name=f"adps_{lt}_{ka}")
                        for k in range(ka, kb):
                            j = k - ka
                            nc.tensor.matmul(adps[:, j * H1:(j + 1) * H1],
                                             lhsT=indn[:, k * P:(k + 1) * P],
                                             rhs=adt,
                                             start=True, stop=True)

                        # es = as_src + ad_dst for all chunks of the part
                        es = sbb.tile([P, KA * H1], F32, tag="es", bufs=3)
                        nc.vector.tensor_tensor(
                            out=es[:, 0:kw * H1].rearrange(
                                "p (k w) -> p k w", w=H1),
                            in0=gv[:, :, H1 + D1:H1 + D1 + H1],
                            in1=adps[:, 0:kw * H1].rearrange(
                                "p (k w) -> p k w", w=H1),
                            op=mybir.AluOpType.add)
                        # el = lrelu(es); ee = exp(el) (contiguous, batched)
                        el = sbb.tile([P, KA * H1], F32, tag="el", bufs=3)
                        nc.vector.tensor_scalar_mul(el[:, 0:kw * H1],
                                                    es[:, 0:kw * H1], NEG)
                        nc.vector.tensor_tensor(
                            out=el[:, 0:kw * H1], in0=el[:, 0:kw * H1],
                            in1=es[:, 0:kw * H1], op=mybir.AluOpType.max)
                        # stt = [el_hi | el_lo | exp(el)] per chunk: ONE
                        # accumulation group per PSUM bank (start= clears the
                        # whole bank's has_written bits, so interleaved groups
                        # in one bank corrupt each other)
                        stt = sbb.tile([P, KA * 3 * H1], BF16, tag="stt", bufs=3)
                        sttv = stt[:, 0:kw * 3 * H1].rearrange(
                            "p (k w) -> p k w", w=3 * H1)
                        elv = el[:, 0:kw * H1].rearrange(
                            "p (k w) -> p k w", w=H1)
                        nc.vector.tensor_copy(out=sttv[:, :, 0:H1], in_=elv)
                        # low part of el (bf16 rounding residual) so the raw
                        # score sums S reach f32 accuracy in PSUM
                        nc.vector.tensor_tensor(
                            out=sttv[:, :, H1:2 * H1], in0=elv,
                            in1=sttv[:, :, 0:H1], op=mybir.AluOpType.subtract)
                        nc.scalar.activation(sttv[:, :, 2 * H1:3 * H1], elv,
                                             mybir.ActivationFunctionType.Exp)
                        # ee16: each exp weight replicated x16 so the msg
                        # multiply below has step-1 APs on BOTH operands
                        # (DVE 2x_1P mode instead of 1x with a 0-step AP)
                        ee16 = sbb.tile([P, KA * H1 * 16], BF16, tag="ee16", bufs=3)
                        e16v = ee16[:, 0:kw * H1 * 16].rearrange(
                            "p (k h i) -> p k h i", h=H1, i=16)
                        nc.vector.tensor_copy(
                            out=e16v,
                            in_=sttv[:, :, 2 * H1:3 * H1].rearrange(
                                "p k (h o) -> p k h o", o=1)
                                .to_broadcast([P, kw, H1, 16]))
                        # msg = h_src * exp, broadcast per head (batched)
                        msg = sbb.tile([P, KA * D1], BF16, tag="msg", bufs=3)
                        nc.vector.tensor_tensor(
                            out=msg[:, 0:kw * D1].rearrange(
                                "p (k h r i) -> p k h r i", h=H1, r=8, i=16),
                            in0=gv[:, :, H1:H1 + D1].rearrange(
                                "p k (h r i) -> p k h r i", h=H1, i=16),
                            in1=e16v.rearrange("p k h (o i) -> p k h o i", o=1)
                                .to_broadcast([P, kw, H1, 8, 16]),
                            op=mybir.AluOpType.mult)

                        for k in range(ka, kb):
                            j = k - ka
                            ind = ind_en[:, (lt * K + k) * P:(lt * K + k + 1) * P]
                            first, last = k == 0, k == K - 1
                            mo = j * D1
                            nc.tensor.matmul(pab[:, 0:512], lhsT=ind,
                                             rhs=msg[:, mo:mo + 512],
                                             start=first, stop=last)
                            nc.tensor.matmul(pab[:, 512:1024], lhsT=ind,
                                             rhs=msg[:, mo + 512:mo + 1024],
                                             start=first, stop=last)
                            nc.tensor.matmul(ps_s[:], lhsT=ind,
                                             rhs=stt[:, j * 3 * H1:
                                                     (j + 1) * 3 * H1],
                                             start=first, stop=last)

                    # epilogue: denom = T + exp(S + ln 1e-16); normalize, bias,
                    # ELU, transpose, W2 matmul
                    shl = sbe.tile([P, 2 * H1], F32, tag="shl")
                    nc.vector.tensor_copy(out=shl[:], in_=ps_s[:, 0:2 * H1])
                    ssum = sbe.tile([P, H1], F32, tag="ssum")
                    nc.vector.tensor_tensor(out=ssum[:], in0=shl[:, 0:H1],
                                            in1=shl[:, H1:2 * H1],
                                            op=mybir.AluOpType.add)
                    dd = sbe.tile([P, H1], F32, tag="dd")
                    nc.scalar.activation(dd[:], ssum[:],
                                         mybir.ActivationFunctionType.Exp,
                                         bias=lncst[:])
                    nc.vector.tensor_tensor(out=dd[:], in0=dd[:],
                                            in1=ps_s[:, 2 * H1:3 * H1],
                                            op=mybir.AluOpType.add)
                    rr = sbe.tile([P, H1], F32, tag="rr")
                    nc.vector.reciprocal(rr[:], dd[:])
                    h1 = sbe.tile([P, D1], F32, tag="h1")
                    nc.vector.tensor_tensor(
                        out=h1[:, 0:512].rearrange("p (h c) -> p h c", h=4),
                        in0=pab[:, 0:512].rearrange("p (h c) -> p h c", h=4),
                        in1=rr[:, 0:4].rearrange("p (h o) -> p h o", o=1)
                            .to_broadcast([P, 4, C1]),
                        op=mybir.AluOpType.mult)
                    nc.vector.tensor_tensor(
                        out=h1[:, 512:1024].rearrange("p (h c) -> p h c", h=4),
                        in0=pab[:, 512:1024].rearrange("p (h c) -> p h c", h=4),
                        in1=rr[:, 4:8].rearrange("p (h o) -> p h o", o=1)
                            .to_broadcast([P, 4, C1]),
                        op=mybir.AluOpType.mult)
                    nc.vector.tensor_tensor(out=h1[:], in0=h1[:], in1=b1_sb[:],
                                            op=mybir.AluOpType.add)
                    # ELU: out = exp(x - relu(x)) + relu(x) - 1
                    hr = sbe.tile([P, D1], F32, tag="hr")
                    nc.scalar.activation(hr[:], h1[:],
                                         mybir.ActivationFunctionType.Relu)
                    hm = sbe.tile([P, D1], F32, tag="hm")
                    nc.vector.tensor_tensor(out=hm[:], in0=h1[:], in1=hr[:],
                                            op=mybir.AluOpType.subtract)
                    he = sbe.tile([P, D1], F32, tag="he")
                    nc.scalar.activation(he[:], hm[:],
                                         mybir.ActivationFunctionType.Exp)
                    nc.vector.tensor_tensor(out=hm[:], in0=he[:], in1=hr[:],
                                            op=mybir.AluOpType.add)
                    heb = sbe.tile([P, D1], BF16, tag="heb")
                    nc.scalar.activation(heb[:], hm[:],
                                         mybir.ActivationFunctionType.Identity,
                                         bias=m1cst[:])
                    # transpose he -> ht [ch, node] slices, all bf16 (copies
                    # split across DVE and ACT to balance engine load)
                    ht = sbe.tile([P, D1], BF16, tag="ht")
                    for j in range(8):
                        pt = psmisc.tile([P, P], BF16, tag="misc",
                                         name=f"pt_{lt}_{j}")
                        nc.tensor.transpose(pt[:], in_=heb[:, j * P:(j + 1) * P],
                                            identity=identb[:])
                        if j % 2 == 0:
                            nc.vector.tensor_copy(out=ht[:, j * P:(j + 1) * P],
                                                  in_=pt[:])
                        else:
                            nc.scalar.activation(
                                ht[:, j * P:(j + 1) * P], pt[:],
                                mybir.ActivationFunctionType.Copy)
                    ph2 = psmisc.tile([P, R2W], F32, tag="misc",
                                      name=f"ph2_{lt}")
                    for j in range(8):
                        nc.tensor.matmul(
                            ph2[:], lhsT=ht[:, j * P:(j + 1) * P],
                            rhs=w2_sb[:].rearrange("p (j n) -> p j n", j=8)[:, j, :],
                            start=(j == 0), stop=(j == 7))
                    t0 = lt * T2R
                    nc.vector.memset(h2_sb[:, t0 + R2W:t0 + T2R], 0.0)
                    nc.vector.tensor_copy(out=h2_sb[:, t0:t0 + R2W], in_=ph2[:])
                    nc.vector.tensor_copy(
                        out=ad2_all[:, lt:lt + 1],
                        in_=ph2[:, OUT + 1:OUT + 2])
                nc.sync.dma_start(
                    out=h2loc[:].rearrange("(j p) w -> p j w", p=P),
                    in_=h2_sb[:].rearrange("p (j w) -> p j w", j=TPC))

            # ---- AG2: AllGather the layer-2 table ----
            with nc.named_scope("AG2"):
                nc.gpsimd.collective_compute(
                    "AllGather", mybir.AluOpType.bypass,
                    replica_groups=[list(range(NCORES))],
                    ins=[h2loc.opt()], outs=[tab2.opt()])

            # ---- Phase D: layer-2 aggregation ----
            with nc.named_scope("phD"), \
                 tc.tile_pool(name="phd_sb", bufs=2) as sbd, \
                 tc.tile_pool(name="phd_ps", bufs=2, space="PSUM") as psd:
                if dummy_d:
                    for lt in range(TPC):
                        z = sbd.tile([P, OUT], F32, tag="z")
                        nc.vector.memset(z[:], 0.0)
                        nc.sync.dma_start(
                            out=out_d[lt * P:(lt + 1) * P, :], in_=z[:])
                for lt in range(TPC if not dummy_d else 0):
                    i0 = lt * K * 8
                    po = psd.tile([P, OUT + 16], F32, tag="po")
                    ad2t8 = sbd.tile([P, 8], BF16, tag="ad2t8")
                    nc.vector.tensor_copy(
                        out=ad2t8[:],
                        in_=ad2_all[:, lt:lt + 1].to_broadcast([P, 8]))
                    indn2 = sbd.tile([P, K * P], BF16, tag="indn2")
                    nc.sync.dma_start(out=indn2[:],
                                      in_=ind_ne_d[lt * P:(lt + 1) * P, :])
                    g2 = sbd.tile([P, K * T2R], BF16, tag="g2", bufs=3)
                    for pi, (ka, kb) in enumerate(parts):
                        nc.gpsimd.dma_gather(
                            g2[:, ka * T2R:kb * T2R].rearrange(
                                "p (k w) -> p k w", w=T2R),
                            tab2[:], src16[:, i0 + ka * 8:i0 + kb * 8],
                            (kb - ka) * P, (kb - ka) * P, T2R,
                            queue_num=pi % 2)
                    ad2ps = psd.tile([P, K * 8], F32, tag="ad2ps")
                    for k in range(K):
                        nc.tensor.matmul(ad2ps[:, k * 8:(k + 1) * 8],
                                         lhsT=indn2[:, k * P:(k + 1) * P],
                                         rhs=ad2t8[:], start=True, stop=True)
                    g2v = g2[:].rearrange("p (k w) -> p k w", w=T2R)
                    # es2 = as2_src + ad2_dst, batched over chunks
                    es2 = sbd.tile([P, K], F32, tag="es2")
                    nc.vector.tensor_tensor(
                        out=es2[:].rearrange("p (k o) -> p k o", o=1),
                        in0=g2v[:, :, OUT:OUT + 1],
                        in1=ad2ps[:].rearrange("p (k w) -> p k w", w=8)[:, :, 0:1],
                        op=mybir.AluOpType.add)
                    # el2 = lrelu(es2); ee2 = exp(el2) (contiguous, batched)
                    el2 = sbd.tile([P, K], F32, tag="el2")
                    nc.vector.tensor_scalar_mul(el2[:], es2[:], NEG)
                    nc.vector.tensor_tensor(out=el2[:], in0=el2[:], in1=es2[:],
                                            op=mybir.AluOpType.max)
                    el2b = sbd.tile([P, K], BF16, tag="el2b")
                    nc.vector.tensor_copy(out=el2b[:], in_=el2[:])
                    ee2 = sbd.tile([P, K], BF16, tag="ee2")
                    nc.scalar.activation(ee2[:], el2[:],
                                         mybir.ActivationFunctionType.Exp)
                    # rhs2 = [msg 64 | ee | el] per chunk (single group)
                    rhs2 = sbd.tile([P, K * W2F], BF16, tag="rhs2", bufs=3)
                    r2v = rhs2[:].rearrange("p (k w) -> p k w", w=W2F)
                    nc.vector.tensor_tensor(
                        out=r2v[:, :, 0:OUT],
                        in0=g2v[:, :, 0:OUT],
                        in1=ee2[:].rearrange("p (k o) -> p k o", o=1)
                            .to_broadcast([P, K, OUT]),
                        op=mybir.AluOpType.mult)
                    nc.vector.tensor_copy(
                        out=r2v[:, :, OUT:OUT + 1],
                        in_=ee2[:].rearrange("p (k o) -> p k o", o=1))
                    nc.vector.tensor_copy(
                        out=r2v[:, :, OUT + 1:OUT + 2],
                        in_=el2b[:].rearrange("p (k o) -> p k o", o=1))
                    for k in range(K):
                        first, last = k == 0, k == K - 1
                        ind = ind_en[:, (lt * K + k) * P:(lt * K + k + 1) * P]
                        nc.tensor.matmul(po[:, 0:W2F], lhsT=ind,
                                         rhs=rhs2[:, k * W2F:(k + 1) * W2F],
                                         start=first, stop=last)
                    dd2 = sbd.tile([P, 1], F32, tag="dd2")
                    nc.scalar.activation(dd2[:], po[:, OUT + 1:OUT + 2],
                                         mybir.ActivationFunctionType.Exp,
                                         bias=lncst[:])
                    nc.vector.tensor_tensor(out=dd2[:], in0=dd2[:],
                                            in1=po[:, OUT:OUT + 1],
                                            op=mybir.AluOpType.add)
                    r2 = sbd.tile([P, 1], F32, tag="r2")
                    nc.vector.reciprocal(r2[:], dd2[:])
                    o_sb = sbd.tile([P, OUT], F32, tag="o_sb")
                    nc.vector.tensor_tensor(
                        out=o_sb[:], in0=po[:, 0:OUT],
                        in1=r2[:].to_broadcast([P, OUT]),
                        op=mybir.AluOpType.mult)
                    nc.vector.tensor_tensor(out=o_sb[:], in0=o_sb[:], in1=b2_sb[:],
                                            op=mybir.AluOpType.add)
                    nc.sync.dma_start(out=out_d[lt * P:(lt + 1) * P, :],
                                      in_=o_sb[:])

    nc.compile()
    return nc


_CACHE = {}
TRACE = False          # set by test.py to capture a neuron-profile trace
LAST_EXEC_NS = None
LAST_RESULTS = None


def kernel(x, edge_index, W1, a_src1, a_dst1, b1, W2, a_src2, a_dst2, b2):
    x = np.asarray(x, np.float32)
    edge_index = np.asarray(edge_index)
    W1 = np.asarray(W1, np.float32)
    a_src1 = np.asarray(a_src1, np.float32)
    a_dst1 = np.asarray(a_dst1, np.float32)
    b1 = np.asarray(b1, np.float32)
    W2 = np.asarray(W2, np.float32)
    a_src2 = np.asarray(a_src2, np.float32)
    a_dst2 = np.asarray(a_dst2, np.float32)
    b2 = np.asarray(b2, np.float32)

    K, src16, ind_en, ind_ne = _prep_edges(edge_index)

    # fold attention vectors into the weight matrices (host-side reparam)
    Asrc = np.zeros((D1, H1), np.float32)
    Adst = np.zeros((D1, H1), np.float32)
    for h in range(H1):
        Asrc[h * C1:(h + 1) * C1, h] = a_src1[h]
        Adst[h * C1:(h + 1) * C1, h] = a_dst1[h]
    # htab row layout [ad 8 | h 1024 | as 8]: ad first so the indirect
    # per-node score gather can use an offset-0 source AP
    wext = np.concatenate([W1 @ Adst, W1, W1 @ Asrc], axis=1)       # [128, 1040]
    w2ext = np.concatenate([W2, W2 @ a_src2[0][:, None],
                            W2 @ a_dst2[0][:, None]], axis=1)        # [1024, 66]

    import ml_dtypes
    xT = np.zeros((IN, NP_PAD), np.float32)
    xT[:, :N] = x.T
    xTb = xT.astype(ml_dtypes.bfloat16)
    wextb = wext.astype(ml_dtypes.bfloat16)
    b1b = np.broadcast_to(b1, (P, D1)).copy()
    b2b = np.broadcast_to(b2, (P, OUT)).copy()
    adtidx = np.empty((NCORES, P, TPC * 8), np.int16)
    for c in range(NCORES):
        for lt in range(TPC):
            nodes = (c * NLOC + lt * P + np.arange(P)).astype(np.int16)
            adtidx[c, :, lt * 8:(lt + 1) * 8] = _wrap16(nodes)

    if K not in _CACHE:
        _CACHE[K] = _build_program(K)
    nc = _CACHE[K]

    in_maps = []
    for c in range(NCORES):
        in_maps.append({
            "xTb": xTb,
            "wext": wextb, "w2ext": w2ext.astype(ml_dtypes.bfloat16), "b1b": b1b, "b2b": b2b,
            "src16": src16[c], "adtidx": adtidx[c],
            "ind_en": np.asarray(ind_en[c]), "ind_ne": np.asarray(ind_ne[c]),
        })
    res = run_bass_kernel_spmd(nc, in_maps, list(range(NCORES)), trace=TRACE)
    global LAST_EXEC_NS, LAST_RESULTS
    LAST_EXEC_NS = res.exec_time_ns
    LAST_RESULTS = res
    out = np.concatenate([res.results[c]["out"] for c in range(NCORES)], axis=0)
    return np.ascontiguousarray(out[:N]).astype(np.float32)


# revision 67
# speedup vs baseline: 1.1094x; 1.0062x over previous
"""Two-layer GAT (PyG GATConv semantics) on 8 Trainium2 NeuronCores.

Sharding: dst nodes partitioned into 8 contiguous ranges (graph parallel).
v2b pipeline per core:
  Phase A: compute the full layer-1 feature table h = x @ W1 (bf16, with
           a_src/a_dst scores folded into the same matmul via W1 @ A) for
           ALL nodes, replicated on every core (an AllGather of the 23.6MB
           table is cheaper, but dma_gather from large Shared-space tables
           hangs TRN2).  Rows stored bf16 [ad 8 | h 1024 | as 8 | pad] ->
           htab [10240,1152] in local DRAM, batched 4 tiles per DMA.
  Phase B: for each of the core's 10 dst tiles: gather per-edge source rows
           with dma_gather, compute edge attention (batched), aggregate
           messages + softmax denominators with indicator-matrix matmuls on
           the TensorEngine.  Indicators are host-built; ind_en for ALL
           tiles lives resident in SBUF (one DMA), ind_ne streams per tile.
           Per-tile a_dst scores come from an indirect gather of htab cols
           1032:1040.  Epilogue: normalize, +b1, ELU, transpose, matmul
           with W2_ext -> layer-2 table shard (kept in SBUF + one DMA out).
  AG2:     AllGather of the small layer-2 table (h2 | as2 | ad2).
  Phase D: same gather/aggregate for layer 2 (single head), normalize, +b2.

NOTE on the softmax: the reference's jax.ops.segment_max lowers to a segment
*sum* on this backend, so the executed oracle computes
  alpha = exp(e - S_dst) / (sum(exp(e - S_dst)) + 1e-16),   S = sum(e)
which equals exp(e) / (T + exp(S + ln 1e-16)) with T = sum(exp(e)).
We accumulate both T and S per node and use that denominator.
"""
import numpy as np

import concourse.bass as bass
import concourse.bacc as bacc
import concourse.mybir as mybir
import concourse.tile as tile
from concourse.bass_utils import run_bass_kernel_spmd
from concourse.masks import make_identity
from concourse.tile import TileContext

# Problem constants (hardcoded per the harness contract).
N = 10000
E = 160000
IN = 128
H1, C1 = 8, 128
D1 = H1 * C1          # 1024
OUT = 64
NEG = 0.2             # leaky_relu slope
NCORES = 8
P = 128
NP_PAD = 10240        # padded node count: 80 tiles of 128
TPC = 10              # dst tiles per core
NT_ALL = NP_PAD // P  # 80
NLOC = NP_PAD // NCORES  # 1280

F32 = mybir.dt.float32
BF16 = mybir.dt.bfloat16
I16 = mybir.dt.int16

HW = 1152             # h-table row width (2304B bf16): [h 1024 | as 8 | pad]
SW = 8                # score-table row width: ad scores only
T2R = 128             # layer-2 table row width: [h2 64 | as2 | ad2 | pad]
R2W = OUT + 2         # 66: w2ext cols = [msg 64 | as2 | ad2]
W2F = OUT + 2         # 66: rhs2 cols per chunk = [msg 64 | ee | el]
LN16 = float(np.log(np.float32(1e-16)))


def _wrap16(ix):
    """dma_gather idx layout: position i -> [i % 16, i // 16], the 16-row
    block replicated across the 8 GpSimd cores (128 partitions)."""
    n = ix.shape[0]
    a = ix.reshape(n // 16, 16).T
    return np.tile(a, (8, 1))


def _prep_edges(edge_index):
    """Sort edges (plus self-loops) by dst, bucket into per-dst-tile chunk
    lists padded to a uniform per-tile chunk count K.  Build the gather
    index planes and host-side indicator matrices (batched per-tile)."""
    import ml_dtypes
    src = np.concatenate([edge_index[0], np.arange(N)]).astype(np.int64)
    dst = np.concatenate([edge_index[1], np.arange(N)]).astype(np.int64)
    order = np.argsort(dst, kind="stable")
    src, dst = src[order], dst[order]

    tile_id = (dst // P).astype(np.int64)
    counts = np.bincount(tile_id, minlength=NT_ALL)
    K = int(np.max((counts + P - 1) // P))
    cap = K * P

    starts = np.zeros(NT_ALL + 1, np.int64)
    starts[1:] = np.cumsum(counts)

    src16 = np.zeros((NCORES, P, TPC * K * 8), np.int16)
    dloc = np.full((NCORES, TPC * K, P), -1.0, np.float32)
    for t in range(NT_ALL):
        c, lt = divmod(t, TPC)
        n = counts[t]
        sl = slice(starts[t], starts[t + 1])
        buf_s = np.zeros(cap, np.int16)
        buf_l = np.full(cap, -1.0, np.float32)
        buf_s[:n] = src[sl]
        buf_l[:n] = (dst[sl] - t * P).astype(np.float32)
        src16[c, :, lt * K * 8:(lt + 1) * K * 8] = _wrap16(buf_s)
        dloc[c, lt * K:(lt + 1) * K] = buf_l.reshape(K, P)

    # Host-built indicator matrices, batched per tile.
    #   ind_en[c][p, (lt*K+k)*128 + d] = (dloc[c, lt*K+k, p] == d)
    #   ind_ne[c][lt*128+n, k*128+e]   = (dloc[c, lt*K+k, e] == n)
    iota = np.arange(P, dtype=np.float32)
    ind_en = np.zeros((NCORES, P, TPC * K * P), ml_dtypes.bfloat16)
    ind_ne = np.zeros((NCORES, TPC * P, K * P), ml_dtypes.bfloat16)
    for c in range(NCORES):
        ind = (dloc[c][:, :, None] == iota[None, None, :])  # [TPC*K, e, d]
        # en: [e_part, chunk*128 + d]
        ind_en[c] = ind.transpose(1, 0, 2).reshape(
            P, TPC * K * P).astype(ml_dtypes.bfloat16)
        # ne: [TPC, n, K, e] -> [TPC*128, K*128]
        ine = ind.transpose(0, 2, 1).reshape(TPC, K, P, P)  # [TPC, k, n, e]
        ind_ne[c] = ine.transpose(0, 2, 1, 3).reshape(
            TPC * P, K * P).astype(ml_dtypes.bfloat16)

    return K, src16, ind_en, ind_ne


def _build_program(K):
    import os
    dummy_b = os.environ.get("KVAR_B", "0") == "1"
    dummy_d = os.environ.get("KVAR_D", "0") == "1"
    nc = bacc.Bacc("TRN2", target_bir_lowering=False, debug=False,
                   num_swdge_queues=2)

    xTb_d = nc.declare_dram_parameter("xTb", [IN, NP_PAD], BF16, isOutput=False)
    wext_d = nc.declare_dram_parameter("wext", [IN, D1 + 16], BF16, isOutput=False)
    w2ext_d = nc.declare_dram_parameter("w2ext", [D1, R2W], BF16, isOutput=False)
    b1_d = nc.declare_dram_parameter("b1b", [P, D1], F32, isOutput=False)
    b2_d = nc.declare_dram_parameter("b2b", [P, OUT], F32, isOutput=False)
    ind_en_d = nc.declare_dram_parameter("ind_en", [P, TPC * K * P], BF16,
                                         isOutput=False)
    ind_ne_d = nc.declare_dram_parameter("ind_ne", [TPC * P, K * P], BF16,
                                         isOutput=False)
    src16_d = nc.declare_dram_parameter("src16", [P, TPC * K * 8], I16,
                                        isOutput=False)
    adtidx_d = nc.declare_dram_parameter("adtidx", [P, TPC * 8], I16,
                                         isOutput=False)
    out_d = nc.declare_dram_parameter("out", [NLOC, OUT], F32, isOutput=True)
    kdbg = os.environ.get("KDBG", "0") == "1"
    if kdbg:
        dbg_adt_d = nc.declare_dram_parameter("dbg_adt", [P, H1], F32,
                                              isOutput=True)
        dbg_htab_d = nc.declare_dram_parameter("dbg_htab", [P, 32], F32,
                                               isOutput=True)
        dbg_g_d = nc.declare_dram_parameter("dbg_g", [P, 32], F32,
                                            isOutput=True)

    # parts cap at ~6 chunks (768 rows) per dma_gather call: larger calls
    # (1152+ indices) hang the SWDGE gather ucode on this hardware
    NPART = (K + 7) // 8
    bounds = np.linspace(0, K, NPART + 1).astype(int)
    parts = [(int(bounds[i]), int(bounds[i + 1])) for i in range(NPART)]
    KA = max(b - a for a, b in parts)

    with TileContext(nc) as tc:
        with tc.tile_pool(name="dram", bufs=1, space="DRAM") as dram, \
             tc.tile_pool(name="const", bufs=1) as const:

            htab = dram.tile([NP_PAD, HW], BF16)
            h2loc = dram.tile([NLOC, T2R], BF16)
            tab2 = dram.tile([NP_PAD, T2R], BF16, addr_space="Shared")

            ident = const.tile([P, P], F32)
            make_identity(nc, ident[:])
            identb = const.tile([P, P], BF16)
            nc.vector.tensor_copy(out=identb[:], in_=ident[:])
            src16 = const.tile([P, TPC * K * 8], I16)
            nc.sync.dma_start(out=src16[:], in_=src16_d[:])
            ind_en = const.tile([P, TPC * K * P], BF16)
            nc.sync.dma_start(out=ind_en[:], in_=ind_en_d[:])
            b1_sb = const.tile([P, D1], F32)
            nc.sync.dma_start(out=b1_sb[:], in_=b1_d[:])
            b2_sb = const.tile([P, OUT], F32)
            nc.sync.dma_start(out=b2_sb[:], in_=b2_d[:])
            w2_sb = const.tile([P, 8 * R2W], BF16)
            nc.sync.dma_start(
                out=w2_sb[:].rearrange("p (j n) -> p j n", j=8),
                in_=w2ext_d[:].rearrange("(j p) n -> p j n", p=P),
            )
            lncst = const.tile([P, 1], F32)
            nc.gpsimd.memset(lncst[:], LN16)
            m1cst = const.tile([P, 1], F32)
            nc.gpsimd.memset(m1cst[:], -1.0)
            adtidx = const.tile([P, TPC * 8], I16)
            nc.sync.dma_start(out=adtidx[:], in_=adtidx_d[:])
            adt_all = const.tile([P, TPC * P], BF16)
            # per-node attention-score stashes (filled by phase B)
            ad2_all = const.tile([P, TPC], BF16)
            h2_sb = const.tile([P, TPC * T2R], BF16)

            # ---- Phase A: layer-1 features for ALL nodes (replicated) ----
            GA = 8                       # tiles per store batch
            with nc.named_scope("phA"), \
                 tc.tile_pool(name="pha_sb", bufs=2) as sba, \
                 tc.tile_pool(name="pha_c", bufs=1) as sbac, \
                 tc.tile_pool(name="pha_ps", bufs=2, space="PSUM") as psa:
                wext_sb = sbac.tile([P, D1 + 16], BF16, tag="wext")
                nc.sync.dma_start(out=wext_sb[:], in_=wext_d[:])
                for g in range(NT_ALL // GA):
                    xtb = sba.tile([P, GA * P], BF16, tag="xt", bufs=3)
                    nc.sync.dma_start(out=xtb[:],
                                      in_=xTb_d[:, g * GA * P:(g + 1) * GA * P])
                    hbuf = sba.tile([P, GA * HW], BF16, tag="hbuf", bufs=3)
                    nc.vector.memset(
                        hbuf[:].rearrange("p (j w) -> p j w", w=HW)[:, :, 1040:HW],
                        0.0)
                    for u in range(GA):
                        nt = g * GA + u
                        ph = psa.tile([P, D1 + 16], F32, tag="ph")
                        xs = xtb[:, u * P:(u + 1) * P]
                        nc.tensor.matmul(ph[:, 0:512], lhsT=xs,
                                         rhs=wext_sb[:, 0:512],
                                         start=True, stop=True)
                        nc.tensor.matmul(ph[:, 512:1024], lhsT=xs,
                                         rhs=wext_sb[:, 512:1024],
                                         start=True, stop=True)
                        nc.tensor.matmul(ph[:, 1024:1040], lhsT=xs,
                                         rhs=wext_sb[:, 1024:1040],
                                         start=True, stop=True)
                        o0 = u * HW
                        # split the psum->sbuf cast across DVE and ACT
                        nc.vector.tensor_copy(out=hbuf[:, o0:o0 + 512],
                                              in_=ph[:, 0:512])
                        nc.scalar.activation(hbuf[:, o0 + 512:o0 + 1024],
                                             ph[:, 512:1024],
                                             mybir.ActivationFunctionType.Copy)
                        nc.vector.tensor_copy(out=hbuf[:, o0 + 1024:o0 + 1040],
                                              in_=ph[:, 1024:1040])
                    nc.sync.dma_start(
                        out=htab[g * GA * P:(g + 1) * GA * P, :].rearrange(
                            "(j p) w -> p j w", p=P),
                        in_=hbuf[:].rearrange("p (j w) -> p j w", j=GA))

            # ---- Phase B: layer-1 aggregation + layer-2 table shard ----
            with nc.named_scope("phB"), \
                 tc.tile_pool(name="phb_sb", bufs=2) as sbb, \
                 tc.tile_pool(name="phb_epi", bufs=2) as sbe, \
                 tc.tile_pool(name="phb_ps2", bufs=2, space="PSUM") as psb2, \
                 tc.tile_pool(name="phb_psm", bufs=2, space="PSUM") as psmisc:
                if dummy_b:
                    nc.vector.memset(h2_sb[:], 0.0)
                    nc.vector.memset(ad2_all[:], 0.0)
                # all 10 tiles' a_dst score rows in two up-front gathers so
                # the per-tile adps matmuls never wait mid-stream
                for h in range(2):
                    nc.gpsimd.dma_gather(
                        adt_all[:, h * 5 * P:(h + 1) * 5 * P].rearrange(
                            "p (k w) -> p k w", w=P),
                        htab[:, 0:P], adtidx[:, h * 40:(h + 1) * 40],
                        5 * P, 5 * P, P, elem_step=HW, queue_num=h)
                for lt in range(TPC if not dummy_b else 0):
                    i0 = lt * K * 8
                    pab = psb2.tile([P, 1024], F32, tag="pab")
                    ps_s = psb2.tile([P, 3 * H1], F32, tag="ps_s")

                    indn = sbb.tile([P, K * P], BF16, tag="indn", bufs=3)
                    nc.sync.dma_start(out=indn[:],
                                      in_=ind_ne_d[lt * P:(lt + 1) * P, :])
                    adt = adt_all[:, lt * P:lt * P + H1]
                    if kdbg and lt == 0:
                        dv = sbb.tile([P, H1], F32, tag="dbg1")
                        nc.vector.tensor_copy(out=dv[:], in_=adt)
                        nc.sync.dma_start(out=dbg_adt_d[:], in_=dv[:])
                        ht0 = sbb.tile([P, 32], BF16, tag="dbg2")
                        nc.sync.dma_start(out=ht0[:], in_=htab[0:P, 0:32])
                        ht0f = sbb.tile([P, 32], F32, tag="dbg3")
                        nc.vector.tensor_copy(out=ht0f[:], in_=ht0[:])
                        nc.sync.dma_start(out=dbg_htab_d[:], in_=ht0f[:])
                    for pi, (ka, kb) in enumerate(parts):
                        kw = kb - ka
                        g = sbb.tile([P, KA * HW], BF16, tag="g", bufs=3)
                        nc.gpsimd.dma_gather(
                            g[:, 0:kw * HW].rearrange("p (k w) -> p k w", w=HW),
                            htab[:], src16[:, i0 + ka * 8:i0 + kb * 8],
                            kw * P, kw * P, HW, queue_num=pi % 2)
                        gv = g[:, 0:kw * HW].rearrange("p (k w) -> p k w", w=HW)
                        if kdbg and lt == 0 and ka == 0:
                            gf = sbb.tile([P, 32], F32, tag="dbg4")
                            nc.vector.tensor_copy(out=gf[:], in_=g[:, 0:32])
                            nc.sync.dma_start(out=dbg_g_d[:], in_=gf[:])

                        adps = psmisc.tile([P, KA * H1], F32, tag="misc",
                                           name=f"adps_{lt}_{ka}")
                        for k in range(ka, kb):
                            j = k - ka
                            nc.tensor.matmul(adps[:, j * H1:(j + 1) * H1],
                                             lhsT=indn[:, k * P:(k + 1) * P],
                                             rhs=adt,
                                             start=True, stop=True)

                        # es = as_src + ad_dst for all chunks of the part
                        es = sbb.tile([P, KA * H1], F32, tag="es", bufs=3)
                        nc.vector.tensor_tensor(
                            out=es[:, 0:kw * H1].rearrange(
                                "p (k w) -> p k w", w=H1),
                            in0=gv[:, :, H1 + D1:H1 + D1 + H1],
                            in1=adps[:, 0:kw * H1].rearrange(
                                "p (k w) -> p k w", w=H1),
                            op=mybir.AluOpType.add)
                        # el = lrelu(es); ee = exp(el) (contiguous, batched)
                        el = sbb.tile([P, KA * H1], F32, tag="el", bufs=3)
                        nc.vector.tensor_scalar_mul(el[:, 0:kw * H1],
                                                    es[:, 0:kw * H1], NEG)
                        nc.vector.tensor_tensor(
                            out=el[:, 0:kw * H1], in0=el[:, 0:kw * H1],
                            in1=es[:, 0:kw * H1], op=mybir.AluOpType.max)
                        # stt = [el_hi | el_lo | exp(el)] per chunk: ONE
                        # accumulation group per PSUM bank (start= clears the
                        # whole bank's has_written bits, so interleaved groups
                        # in one bank corrupt each other)
                        stt = sbb.tile([P, KA * 3 * H1], BF16, tag="stt", bufs=3)
                        sttv = stt[:, 0:kw * 3 * H1].rearrange(
                            "p (k w) -> p k w", w=3 * H1)
                        elv = el[:, 0:kw * H1].rearrange(
                            "p (k w) -> p k w", w=H1)
                        nc.vector.tensor_copy(out=sttv[:, :, 0:H1], in_=elv)
                        # low part of el (bf16 rounding residual) so the raw
                        # score sums S reach f32 accuracy in PSUM
                        nc.vector.tensor_tensor(
                            out=sttv[:, :, H1:2 * H1], in0=elv,
                            in1=sttv[:, :, 0:H1], op=mybir.AluOpType.subtract)
                        nc.scalar.activation(sttv[:, :, 2 * H1:3 * H1], elv,
                                             mybir.ActivationFunctionType.Exp)
                        # ee16: each exp weight replicated x16 so the msg
                        # multiply below has step-1 APs on BOTH operands
                        # (DVE 2x_1P mode instead of 1x with a 0-step AP)
                        ee16 = sbb.tile([P, KA * H1 * 16], BF16, tag="ee16", bufs=3)
                        e16v = ee16[:, 0:kw * H1 * 16].rearrange(
                            "p (k h i) -> p k h i", h=H1, i=16)
                        nc.vector.tensor_copy(
                            out=e16v,
                            in_=sttv[:, :, 2 * H1:3 * H1].rearrange(
                                "p k (h o) -> p k h o", o=1)
                                .to_broadcast([P, kw, H1, 16]))
                        # msg = h_src * exp, broadcast per head (batched)
                        msg = sbb.tile([P, KA * D1], BF16, tag="msg", bufs=3)
                        nc.vector.tensor_tensor(
                            out=msg[:, 0:kw * D1].rearrange(
                                "p (k h r i) -> p k h r i", h=H1, r=8, i=16),
                            in0=gv[:, :, H1:H1 + D1].rearrange(
                                "p k (h r i) -> p k h r i", h=H1, i=16),
                            in1=e16v.rearrange("p k h (o i) -> p k h o i", o=1)
                                .to_broadcast([P, kw, H1, 8, 16]),
                            op=mybir.AluOpType.mult)

                        for k in range(ka, kb):
                            j = k - ka
                            ind = ind_en[:, (lt * K + k) * P:(lt * K + k + 1) * P]
                            first, last = k == 0, k == K - 1
                            mo = j * D1
                            nc.tensor.matmul(pab[:, 0:512], lhsT=ind,
                                             rhs=msg[:, mo:mo + 512],
                                             start=first, stop=last)
                            nc.tensor.matmul(pab[:, 512:1024], lhsT=ind,
                                             rhs=msg[:, mo + 512:mo + 1024],
                                             start=first, stop=last)
                            nc.tensor.matmul(ps_s[:], lhsT=ind,
                                             rhs=stt[:, j * 3 * H1:
                                                     (j + 1) * 3 * H1],
                                             start=first, stop=last)

                    # epilogue: denom = T + exp(S + ln 1e-16); normalize, bias,
                    # ELU, transpose, W2 matmul
                    shl = sbe.tile([P, 2 * H1], F32, tag="shl")
                    nc.vector.tensor_copy(out=shl[:], in_=ps_s[:, 0:2 * H1])
                    ssum = sbe.tile([P, H1], F32, tag="ssum")
                    nc.vector.tensor_tensor(out=ssum[:], in0=shl[:, 0:H1],
                                            in1=shl[:, H1:2 * H1],
                                            op=mybir.AluOpType.add)
                    dd = sbe.tile([P, H1], F32, tag="dd")
                    nc.scalar.activation(dd[:], ssum[:],
                                         mybir.ActivationFunctionType.Exp,
                                         bias=lncst[:])
                    nc.vector.tensor_tensor(out=dd[:], in0=dd[:],
                                            in1=ps_s[:, 2 * H1:3 * H1],
                                            op=mybir.AluOpType.add)
                    rr = sbe.tile([P, H1], F32, tag="rr")
                    nc.vector.reciprocal(rr[:], dd[:])
                    h1 = sbe.tile([P, D1], F32, tag="h1")
                    nc.vector.tensor_tensor(
                        out=h1[:, 0:512].rearrange("p (h c) -> p h c", h=4),
                        in0=pab[:, 0:512].rearrange("p (h c) -> p h c", h=4),
                        in1=rr[:, 0:4].rearrange("p (h o) -> p h o", o=1)
                            .to_broadcast([P, 4, C1]),
                        op=mybir.AluOpType.mult)
                    nc.vector.tensor_tensor(
                        out=h1[:, 512:1024].rearrange("p (h c) -> p h c", h=4),
                        in0=pab[:, 512:1024].rearrange("p (h c) -> p h c", h=4),
                        in1=rr[:, 4:8].rearrange("p (h o) -> p h o", o=1)
                            .to_broadcast([P, 4, C1]),
                        op=mybir.AluOpType.mult)
                    nc.vector.tensor_tensor(out=h1[:], in0=h1[:], in1=b1_sb[:],
                                            op=mybir.AluOpType.add)
                    # ELU: out = exp(x - relu(x)) + relu(x) - 1
                    hr = sbe.tile([P, D1], F32, tag="hr")
                    nc.scalar.activation(hr[:], h1[:],
                                         mybir.ActivationFunctionType.Relu)
                    hm = sbe.tile([P, D1], F32, tag="hm")
                    nc.vector.tensor_tensor(out=hm[:], in0=h1[:], in1=hr[:],
                                            op=mybir.AluOpType.subtract)
                    he = sbe.tile([P, D1], F32, tag="he")
                    nc.scalar.activation(he[:], hm[:],
                                         mybir.ActivationFunctionType.Exp)
                    nc.vector.tensor_tensor(out=hm[:], in0=he[:], in1=hr[:],
                                            op=mybir.AluOpType.add)
                    heb = sbe.tile([P, D1], BF16, tag="heb")
                    nc.scalar.activation(heb[:], hm[:],
                                         mybir.ActivationFunctionType.Identity,
                                         bias=m1cst[:])
                    # transpose he -> ht [ch, node] slices, all bf16 (copies
                    # split across DVE and ACT to balance engine load)
                    ht = sbe.tile([P, D1], BF16, tag="ht")
                    for j in range(8):
                        pt = psmisc.tile([P, P], BF16, tag="misc",
                                         name=f"pt_{lt}_{j}")
                        nc.tensor.transpose(pt[:], in_=heb[:, j * P:(j + 1) * P],
                                            identity=identb[:])
                        if j % 2 == 0:
                            nc.vector.tensor_copy(out=ht[:, j * P:(j + 1) * P],
                                                  in_=pt[:])
                        else:
                            nc.scalar.activation(
                                ht[:, j * P:(j + 1) * P], pt[:],
                                mybir.ActivationFunctionType.Copy)
                    ph2 = psmisc.tile([P, R2W], F32, tag="misc",
                                      name=f"ph2_{lt}")
                    for j in range(8):
                        nc.tensor.matmul(
                            ph2[:], lhsT=ht[:, j * P:(j + 1) * P],
                            rhs=w2_sb[:].rearrange("p (j n) -> p j n", j=8)[:, j, :],
                            start=(j == 0), stop=(j == 7))
                    t0 = lt * T2R
                    nc.vector.memset(h2_sb[:, t0 + R2W:t0 + T2R], 0.0)
                    nc.vector.tensor_copy(out=h2_sb[:, t0:t0 + R2W], in_=ph2[:])
                    nc.vector.tensor_copy(
                        out=ad2_all[:, lt:lt + 1],
                        in_=ph2[:, OUT + 1:OUT + 2])
                nc.sync.dma_start(
                    out=h2loc[:].rearrange("(j p) w -> p j w", p=P),
                    in_=h2_sb[:].rearrange("p (j w) -> p j w", j=TPC))

            # ---- AG2: AllGather the layer-2 table ----
            with nc.named_scope("AG2"):
                nc.gpsimd.collective_compute(
                    "AllGather", mybir.AluOpType.bypass,
                    replica_groups=[list(range(NCORES))],
                    ins=[h2loc.opt()], outs=[tab2.opt()])

            # ---- Phase D: layer-2 aggregation ----
            with nc.named_scope("phD"), \
                 tc.tile_pool(name="phd_sb", bufs=2) as sbd, \
                 tc.tile_pool(name="phd_ps", bufs=2, space="PSUM") as psd:
                if dummy_d:
                    for lt in range(TPC):
                        z = sbd.tile([P, OUT], F32, tag="z")
                        nc.vector.memset(z[:], 0.0)
                        nc.sync.dma_start(
                            out=out_d[lt * P:(lt + 1) * P, :], in_=z[:])
                for lt in range(TPC if not dummy_d else 0):
                    i0 = lt * K * 8
                    po = psd.tile([P, OUT + 16], F32, tag="po")
                    ad2t8 = sbd.tile([P, 8], BF16, tag="ad2t8", bufs=3)
                    nc.vector.tensor_copy(
                        out=ad2t8[:],
                        in_=ad2_all[:, lt:lt + 1].to_broadcast([P, 8]))
                    indn2 = sbd.tile([P, K * P], BF16, tag="indn2", bufs=3)
                    nc.sync.dma_start(out=indn2[:],
                                      in_=ind_ne_d[lt * P:(lt + 1) * P, :])
                    g2 = sbd.tile([P, K * T2R], BF16, tag="g2", bufs=3)
                    for pi, (ka, kb) in enumerate(parts):
                        nc.gpsimd.dma_gather(
                            g2[:, ka * T2R:kb * T2R].rearrange(
                                "p (k w) -> p k w", w=T2R),
                            tab2[:], src16[:, i0 + ka * 8:i0 + kb * 8],
                            (kb - ka) * P, (kb - ka) * P, T2R,
                            queue_num=pi % 2)
                    ad2ps = psd.tile([P, K * 8], F32, tag="ad2ps")
                    for k in range(K):
                        nc.tensor.matmul(ad2ps[:, k * 8:(k + 1) * 8],
                                         lhsT=indn2[:, k * P:(k + 1) * P],
                                         rhs=ad2t8[:], start=True, stop=True)
                    g2v = g2[:].rearrange("p (k w) -> p k w", w=T2R)
                    # es2 = as2_src + ad2_dst, batched over chunks
                    es2 = sbd.tile([P, K], F32, tag="es2", bufs=3)
                    nc.vector.tensor_tensor(
                        out=es2[:].rearrange("p (k o) -> p k o", o=1),
                        in0=g2v[:, :, OUT:OUT + 1],
                        in1=ad2ps[:].rearrange("p (k w) -> p k w", w=8)[:, :, 0:1],
                        op=mybir.AluOpType.add)
                    # el2 = lrelu(es2); ee2 = exp(el2) (contiguous, batched)
                    el2 = sbd.tile([P, K], F32, tag="el2", bufs=3)
                    nc.vector.tensor_scalar_mul(el2[:], es2[:], NEG)
                    nc.vector.tensor_tensor(out=el2[:], in0=el2[:], in1=es2[:],
                                            op=mybir.AluOpType.max)
                    el2b = sbd.tile([P, K], BF16, tag="el2b", bufs=3)
                    nc.vector.tensor_copy(out=el2b[:], in_=el2[:])
                    ee2 = sbd.tile([P, K], BF16, tag="ee2", bufs=3)
                    nc.scalar.activation(ee2[:], el2[:],
                                         mybir.ActivationFunctionType.Exp)
                    # rhs2 = [msg 64 | ee | el] per chunk (single group)
                    rhs2 = sbd.tile([P, K * W2F], BF16, tag="rhs2", bufs=3)
                    r2v = rhs2[:].rearrange("p (k w) -> p k w", w=W2F)
                    nc.vector.tensor_tensor(
                        out=r2v[:, :, 0:OUT],
                        in0=g2v[:, :, 0:OUT],
                        in1=ee2[:].rearrange("p (k o) -> p k o", o=1)
                            .to_broadcast([P, K, OUT]),
                        op=mybir.AluOpType.mult)
                    nc.vector.tensor_copy(
                        out=r2v[:, :, OUT:OUT + 1],
                        in_=ee2[:].rearrange("p (k o) -> p k o", o=1))
                    nc.vector.tensor_copy(
                        out=r2v[:, :, OUT + 1:OUT + 2],
                        in_=el2b[:].rearrange("p (k o) -> p k o", o=1))
                    for k in range(K):
                        first, last = k == 0, k == K - 1
                        ind = ind_en[:, (lt * K + k) * P:(lt * K + k + 1) * P]
                        nc.tensor.matmul(po[:, 0:W2F], lhsT=ind,
                                         rhs=rhs2[:, k * W2F:(k + 1) * W2F],
                                         start=first, stop=last)
                    dd2 = sbd.tile([P, 1], F32, tag="dd2")
                    nc.scalar.activation(dd2[:], po[:, OUT + 1:OUT + 2],
                                         mybir.ActivationFunctionType.Exp,
                                         bias=lncst[:])
                    nc.vector.tensor_tensor(out=dd2[:], in0=dd2[:],
                                            in1=po[:, OUT:OUT + 1],
                                            op=mybir.AluOpType.add)
                    r2 = sbd.tile([P, 1], F32, tag="r2")
                    nc.vector.reciprocal(r2[:], dd2[:])
                    o_sb = sbd.tile([P, OUT], F32, tag="o_sb")
                    nc.vector.tensor_tensor(
                        out=o_sb[:], in0=po[:, 0:OUT],
                        in1=r2[:].to_broadcast([P, OUT]),
                        op=mybir.AluOpType.mult)
                    nc.vector.tensor_tensor(out=o_sb[:], in0=o_sb[:], in1=b2_sb[:],
                                            op=mybir.AluOpType.add)
                    nc.sync.dma_start(out=out_d[lt * P:(lt + 1) * P, :],
                                      in_=o_sb[:])

    nc.compile()
    return nc


_CACHE = {}
TRACE = False          # set by test.py to capture a neuron-profile trace
LAST_EXEC_NS = None
LAST_RESULTS = None


def kernel(x, edge_index, W1, a_src1, a_dst1, b1, W2, a_src2, a_dst2, b2):
    x = np.asarray(x, np.float32)
    edge_index = np.asarray(edge_index)
    W1 = np.asarray(W1, np.float32)
    a_src1 = np.asarray(a_src1, np.float32)
    a_dst1 = np.asarray(a_dst1, np.float32)
    b1 = np.asarray(b1, np.float32)
    W2 = np.asarray(W2, np.float32)
    a_src2 = np.asarray(a_src2, np.float32)
    a_dst2 = np.asarray(a_dst2, np.float32)
    b2 = np.asarray(b2, np.float32)

    K, src16, ind_en, ind_ne = _prep_edges(edge_index)

    # fold attention vectors into the weight matrices (host-side reparam)
    Asrc = np.zeros((D1, H1), np.float32)
    Adst = np.zeros((D1, H1), np.float32)
    for h in range(H1):
        Asrc[h * C1:(h + 1) * C1, h] = a_src1[h]
        Adst[h * C1:(h + 1) * C1, h] = a_dst1[h]
    # htab row layout [ad 8 | h 1024 | as 8]: ad first so the indirect
    # per-node score gather can use an offset-0 source AP
    wext = np.concatenate([W1 @ Adst, W1, W1 @ Asrc], axis=1)       # [128, 1040]
    w2ext = np.concatenate([W2, W2 @ a_src2[0][:, None],
                            W2 @ a_dst2[0][:, None]], axis=1)        # [1024, 66]

    import ml_dtypes
    xT = np.zeros((IN, NP_PAD), np.float32)
    xT[:, :N] = x.T
    xTb = xT.astype(ml_dtypes.bfloat16)
    wextb = wext.astype(ml_dtypes.bfloat16)
    b1b = np.broadcast_to(b1, (P, D1)).copy()
    b2b = np.broadcast_to(b2, (P, OUT)).copy()
    adtidx = np.empty((NCORES, P, TPC * 8), np.int16)
    for c in range(NCORES):
        for lt in range(TPC):
            nodes = (c * NLOC + lt * P + np.arange(P)).astype(np.int16)
            adtidx[c, :, lt * 8:(lt + 1) * 8] = _wrap16(nodes)

    if K not in _CACHE:
        _CACHE[K] = _build_program(K)
    nc = _CACHE[K]

    in_maps = []
    for c in range(NCORES):
        in_maps.append({
            "xTb": xTb,
            "wext": wextb, "w2ext": w2ext.astype(ml_dtypes.bfloat16), "b1b": b1b, "b2b": b2b,
            "src16": src16[c], "adtidx": adtidx[c],
            "ind_en": np.asarray(ind_en[c]), "ind_ne": np.asarray(ind_ne[c]),
        })
    res = run_bass_kernel_spmd(nc, in_maps, list(range(NCORES)), trace=TRACE)
    global LAST_EXEC_NS, LAST_RESULTS
    LAST_EXEC_NS = res.exec_time_ns
    LAST_RESULTS = res
    out = np.concatenate([res.results[c]["out"] for c in range(NCORES)], axis=0)
    return np.ascontiguousarray(out[:N]).astype(np.float32)
